# revision 11
# baseline (speedup 1.0000x reference)
"""Causal multi-head attention layer for Trainium2, 8-core data-parallel.

Problem: B=8, L=2048, D=512, H=8 heads (dh=64), fp32, causal softmax attention
with QKV/out projections + biases.

Sharding: pure data parallel over batch (B=8 == n_cores); each core runs one
batch element end-to-end; no collectives.

Per-core dataflow (all matmuls contract over the SBUF partition dim):
  - PE-transpose x (q,k,v inputs) into xT [d, tok]
  - QT[dout, tok] = Wq.T chunks @ xT    (bias via K=1 ones-matmul)
  - KT[dout, tok] likewise; V[tok, dout] = xT.T chunks @ Wv (natural layout),
    stored head-interleaved with an appended ones column: [tok, h, 64+1]
  - scores transposed: sT[k, q] = KT_h.T @ QT_h  (K=dh=64), causal-trimmed
  - P = exp(sT/8) on ACT; triangular mask multiply on diagonal blocks (DVE)
  - outT[65, q] += [V_h | 1].T @ P  accumulated over k chunks; row 64 = softmax
    denominators (the ones column) -- no separate reduction needed
  - normalize: r = 1/sums (DVE reciprocal); broadcast r over 64 partitions via
    K=1 matmul; DVE multiply into the staging buffer stage[dk, q]
  - out-proj: O[q, dout] = stage_chunks.T @ Wo chunks + bo (K=1 ones-matmul),
    already in natural layout for the output DMA.
"""

import numpy as np

B = 8
L = 2048
D = 512
H = 8
DH = 64
NT = L // 128   # 16 token tiles
NCH = D // 128  # 4 dim chunks
NQ = L // 512   # 4 q supertiles

_cached = {}


def _build():
    import concourse.bass as bass
    import concourse.tile as tile
    from concourse import mybir, bacc
    from concourse.masks import make_identity

    f32 = mybir.dt.float32
    f32r = mybir.dt.float32r

    nc = bacc.Bacc("TRN2", target_bir_lowering=False, debug=False)

    xq = nc.dram_tensor("query", [L, D], f32, kind="ExternalInput").ap()
    xk = nc.dram_tensor("key", [L, D], f32, kind="ExternalInput").ap()
    xv = nc.dram_tensor("value", [L, D], f32, kind="ExternalInput").ap()
    Wq = nc.dram_tensor("Wq", [D, D], f32, kind="ExternalInput").ap()
    Wk = nc.dram_tensor("Wk", [D, D], f32, kind="ExternalInput").ap()
    Wv = nc.dram_tensor("Wv", [D, D], f32, kind="ExternalInput").ap()
    Wo = nc.dram_tensor("Wo", [D, D], f32, kind="ExternalInput").ap()
    bq = nc.dram_tensor("bq", [D], f32, kind="ExternalInput").ap()
    bk = nc.dram_tensor("bk", [D], f32, kind="ExternalInput").ap()
    bv = nc.dram_tensor("bv", [D], f32, kind="ExternalInput").ap()
    bo = nc.dram_tensor("bo", [D], f32, kind="ExternalInput").ap()
    out = nc.dram_tensor("out", [L, D], f32, kind="ExternalOutput").ap()

    def r(ap):
        return ap.bitcast(f32r)

    with tile.TileContext(nc) as tc:
        with (
            tc.tile_pool(name="persist", bufs=1) as persist,
            tc.tile_pool(name="consts", bufs=1) as consts,
        ):
            # ---- constants ----
            ident = consts.tile([128, 128], f32, tag="ident")
            make_identity(nc, ident[:])
            # tri[x, y] = 1.0 where y >= x else 0  (valid = q_local >= k_local)
            tri = consts.tile([128, 128], f32, tag="tri")
            nc.gpsimd.memset(tri[:], 0.0)
            nc.gpsimd.affine_select(
                out=tri[:], in_=tri[:], compare_op=mybir.AluOpType.is_gt,
                fill=1.0, base=0, pattern=[[-1, 128]], channel_multiplier=1,
            )
            # tri_wide[x, y] (y in [0,256)): 1.0 where y >= x + 128 else 0
            tri_wide = consts.tile([128, 256], f32, tag="tri_wide")
            nc.gpsimd.memset(tri_wide[:], 0.0)
            nc.gpsimd.affine_select(
                out=tri_wide[:], in_=tri_wide[:], compare_op=mybir.AluOpType.is_gt,
                fill=1.0, base=128, pattern=[[-1, 256]], channel_multiplier=1,
            )
            ones = consts.tile([1, 512], f32, tag="ones")
            nc.vector.memset(ones[:], 1.0)
            ones_t = consts.tile([128, 64], mybir.dt.bfloat16, tag="ones_t")
            nc.vector.memset(ones_t[:], 1.0)

            # ---- weights / biases (DMA to temp, round-copy to f32r) ----
            w_sb = {}
            b_sb = {}
            with tc.tile_pool(name="wtmp", bufs=3) as wtmp_pool:
                for name, wdram in (("q", Wq), ("k", Wk), ("v", Wv), ("o", Wo)):
                    t = persist.tile([128, NCH, 512], f32, tag=f"W{name}",
                                     name=f"W{name}")
                    for c in range(NCH):
                        wt = wtmp_pool.tile([128, 512], f32, tag="wtmp", name="wtmp")
                        nc.sync.dma_start(
                            wt[:], wdram[128 * c:128 * (c + 1), :])
                        nc.vector.tensor_copy(r(t[:, c, :]), wt[:])
                    w_sb[name] = t
                for name, bdram in (("q", bq), ("k", bk), ("v", bv), ("o", bo)):
                    t = persist.tile([1, 512], f32, tag=f"b{name}", name=f"b{name}")
                    bt = wtmp_pool.tile([1, 512], f32, tag="btmp", name="btmp",
                                        bufs=2)
                    nc.sync.dma_start(bt[:], bdram[None, :])
                    nc.vector.tensor_copy(r(t[:]), bt[:])
                    b_sb[name] = t

            # ---- persistent activations ----
            qt_sb = [persist.tile([128, L], f32, tag=f"QT{c}", name=f"QT{c}") for c in range(NCH)]
            kt_sb = [persist.tile([128, L], f32, tag=f"KT{c}", name=f"KT{c}") for c in range(NCH)]
            v_sb = [persist.tile([128, H, DH + 1], f32, tag=f"V{t}", name=f"V{t}") for t in range(NT)]
            stage = [persist.tile([128, L], f32, tag=f"stage{c}", name=f"stage{c}") for c in range(NCH)]

            # ================= Phase A: transposes + projections =============
            with (
                tc.tile_pool(name="xin", bufs=6) as xin_pool,
                tc.tile_pool(name="trps", bufs=3, space="PSUM") as trps_pool,
                tc.tile_pool(name="xt", bufs=8) as xt_pool,
                tc.tile_pool(name="projps", bufs=3, space="PSUM") as projps_pool,
            ):
                for tname, xdram in (("q", xq), ("k", xk), ("v", xv)):
                    for g in range(NT // 4):  # groups of 4 token tiles
                        xtiles = []
                        for j in range(4):
                            t0 = 4 * g + j
                            xt_in = xin_pool.tile([128, 512], f32, tag="xin", name="xin")
                            nc.sync.dma_start(
                                xt_in[:], xdram[128 * t0:128 * (t0 + 1), :])
                            xtiles.append(xt_in)
                        # transpose to xT [dchunk][128 d, 512 toks]
                        xt_c = []
                        for c in range(NCH):
                            ps = trps_pool.tile([128, 512], f32, tag="trps", name="trps")
                            for j in range(4):
                                nc.tensor.transpose(
                                    ps[:, 128 * j:128 * (j + 1)],
                                    xtiles[j][:, 128 * c:128 * (c + 1)],
                                    ident[:],
                                )
                            sb = xt_pool.tile([128, 512], f32, tag="xt", name="xt")
                            nc.vector.tensor_copy(r(sb[:]), ps[:])
                            xt_c.append(sb)
                        if tname in ("q", "k"):
                            dst = qt_sb if tname == "q" else kt_sb
                            for co in range(NCH):
                                pp = projps_pool.tile([128, 512], f32, tag="projps", name="projps")
                                for ci in range(NCH):
                                    nc.tensor.matmul(
                                        pp[:],
                                        r(w_sb[tname][:, ci, 128 * co:128 * (co + 1)]),
                                        r(xt_c[ci][:]),
                                        start=(ci == 0), stop=False,
                                    )
                                nc.tensor.matmul(
                                    pp[:],
                                    r(b_sb[tname][0:1, 128 * co:128 * (co + 1)]),
                                    r(ones[0:1, :]),
                                    start=False, stop=True,
                                )
                                nc.vector.tensor_copy(
                                    r(dst[co][:, 512 * g:512 * (g + 1)]), pp[:])
                        else:  # v: natural layout per token tile
                            for j in range(4):
                                t0 = 4 * g + j
                                pv = projps_pool.tile([128, 512], f32, tag="projps", name="projps")
                                for ci in range(NCH):
                                    nc.tensor.matmul(
                                        pv[:],
                                        r(xt_c[ci][:, 128 * j:128 * (j + 1)]),
                                        r(w_sb["v"][:, ci, :]),
                                        start=(ci == 0), stop=False,
                                    )
                                nc.tensor.matmul(
                                    pv[:], r(ones[0:1, 0:128]), r(b_sb["v"][0:1, :]),
                                    start=False, stop=True,
                                )
                                nc.vector.tensor_copy(
                                    r(v_sb[t0][:, :, 0:DH]),
                                    pv[:].rearrange("p (h d) -> p h d", h=H),
                                )
                                nc.gpsimd.memset(v_sb[t0][:, :, DH:DH + 1], 1.0)

            # ====== Phase B+C+D: attention, normalize, out-projection ========
            # qt-outer so each q supertile finishes all heads, normalizes, and
            # projects while later supertiles still compute.
            with (
                tc.tile_pool(name="sps", bufs=3, space="PSUM") as sps_pool,
                tc.tile_pool(name="ops", bufs=2, space="PSUM") as ops_pool,
                tc.tile_pool(name="bcps", bufs=1, space="PSUM") as bc_pool,
                tc.tile_pool(name="outps", bufs=2, space="PSUM") as outps_pool,
                tc.tile_pool(name="pexp", bufs=4) as p_pool,
                tc.tile_pool(name="norm", bufs=1) as norm_pool,
                tc.tile_pool(name="osb", bufs=3) as o_pool,
            ):
                for qt in range(NQ):
                    kmax = 4 * qt + 4
                    # sums rows for the 8 heads at 32-aligned partitions:
                    # head h -> partition 32*(h%4), free offset 512*(h//4)
                    stg = norm_pool.tile([128, 1536], f32, tag="stg", name="stg")
                    for h in range(H):
                        ch, prow = h // 2, 64 * (h % 2)
                        kth = kt_sb[ch]
                        qth = qt_sb[ch]
                        po = ops_pool.tile([65, 512], f32, tag="ops", name="ops")
                        for c in range(kmax):
                            m = c - 4 * qt
                            # j0: first valid q col in this 512 window (128-gran),
                            # widened to keep matmul N >= 256 (f32r fast path)
                            j0 = 0 if m < 1 else (128 * m if m < 3 else 256)
                            ps = sps_pool.tile([128, 512], f32, tag="sps", name="sps")
                            nc.tensor.matmul(
                                ps[:, j0:512],
                                r(kth[prow:prow + DH, 128 * c:128 * (c + 1)]),
                                r(qth[prow:prow + DH, 512 * qt + j0:512 * (qt + 1)]),
                                start=True, stop=True,
                            )
                            pt = p_pool.tile([128, 512], f32, tag="pexp", name="pexp")
                            nc.scalar.activation(
                                r(pt[:, j0:512]), ps[:, j0:512],
                                mybir.ActivationFunctionType.Exp, scale=0.125,
                            )
                            if m == 3:
                                nc.vector.tensor_mul(
                                    r(pt[:, 256:512]), pt[:, 256:512], tri_wide[:])
                            elif m >= 0:
                                nc.vector.tensor_mul(
                                    r(pt[:, 128 * m:128 * (m + 1)]),
                                    pt[:, 128 * m:128 * (m + 1)], tri[:])
                            nc.tensor.matmul(
                                po[:, j0:512],
                                r(v_sb[c][:, h, :]),
                                r(pt[:, j0:512]),
                                start=(c == 0), stop=(c == kmax - 1),
                            )
                        nc.vector.tensor_copy(
                            r(stage[ch][prow:prow + DH, 512 * qt:512 * (qt + 1)]),
                            po[0:DH, :])
                        nc.vector.tensor_copy(
                            stg[32 * (h % 3):32 * (h % 3) + 1,
                                512 * (h // 3):512 * (h // 3) + 512],
                            po[DH:DH + 1, :])
                    # reciprocal of all 8 sums rows (unused partitions hold
                    # garbage that is never read)
                    rstg = norm_pool.tile([128, 1536], f32, tag="rstg", name="rstg")
                    rscr = norm_pool.tile([128, 1536], f32, tag="rscr", name="rscr")
                    nc.vector.reciprocal_approx_accurate(
                        out=rstg[:], in_=stg[:], scratch=rscr[:])
                    rbf = norm_pool.tile(
                        [128, 1536], mybir.dt.bfloat16, tag="rbf", name="rbf")
                    nc.vector.tensor_copy(rbf[:], rstg[:])
                    for ch in range(NCH):
                        bc = bc_pool.tile([128, 512], f32, tag="bcps", name="bcps")
                        for sub in range(2):
                            hh = 2 * ch + sub
                            pp0 = 32 * (hh % 3)
                            fo = 512 * (hh // 3)
                            nc.tensor.matmul(
                                bc[64 * sub:64 * sub + 64, :],
                                ones_t[pp0:pp0 + 1, 0:64],
                                rbf[pp0:pp0 + 1, fo:fo + 512],
                                start=True, stop=True,
                            )
                        nc.vector.tensor_mul(
                            r(stage[ch][:, 512 * qt:512 * (qt + 1)]),
                            stage[ch][:, 512 * qt:512 * (qt + 1)],
                            bc[:],
                        )
                    for i in range(4 * qt, 4 * qt + 4):
                        pout = outps_pool.tile(
                            [128, 512], f32, tag="outps", name="outps")
                        for ch in range(NCH):
                            nc.tensor.matmul(
                                pout[:],
                                r(stage[ch][:, 128 * i:128 * (i + 1)]),
                                r(w_sb["o"][:, ch, :]),
                                start=(ch == 0), stop=False,
                            )
                        nc.tensor.matmul(
                            pout[:], r(ones[0:1, 0:128]), r(b_sb["o"][0:1, :]),
                            start=False, stop=True,
                        )
                        ot = o_pool.tile([128, 512], f32, tag="osb", name="osb")
                        nc.vector.tensor_copy(ot[:], pout[:])
                        nc.sync.dma_start(out[128 * i:128 * (i + 1), :], ot[:])

    nc.compile()
    return nc


def get_nc():
    if "nc" not in _cached:
        _cached["nc"] = _build()
    return _cached["nc"]


def run(in_maps, trace=False, **kw):
    from concourse.bass_utils import run_bass_kernel_spmd

    nc = get_nc()
    return run_bass_kernel_spmd(nc, in_maps, list(range(B)), trace=trace, **kw)


def kernel(query, key, value, Wq, bq, Wk, bk, Wv, bv, Wo, bo):
    shared = {
        "Wq": np.ascontiguousarray(Wq, np.float32),
        "Wk": np.ascontiguousarray(Wk, np.float32),
        "Wv": np.ascontiguousarray(Wv, np.float32),
        "Wo": np.ascontiguousarray(Wo, np.float32),
        "bq": np.ascontiguousarray(bq, np.float32),
        "bk": np.ascontiguousarray(bk, np.float32),
        "bv": np.ascontiguousarray(bv, np.float32),
        "bo": np.ascontiguousarray(bo, np.float32),
    }
    in_maps = []
    for i in range(B):
        m = dict(shared)
        m["query"] = np.ascontiguousarray(query[i], np.float32)
        m["key"] = np.ascontiguousarray(key[i], np.float32)
        m["value"] = np.ascontiguousarray(value[i], np.float32)
        in_maps.append(m)
    res = run(in_maps)
    return np.stack([res.results[i]["out"] for i in range(B)], axis=0)


# revision 13
# speedup vs baseline: 1.3773x; 1.3773x over previous
"""Causal multi-head attention layer for Trainium2, 8-core data-parallel.

Problem: B=8, L=2048, D=512, H=8 heads (dh=64), fp32, causal softmax attention
with QKV/out projections + biases.

Sharding: pure data parallel over batch (B=8 == n_cores); each core runs one
batch element end-to-end; no collectives.

Per-core dataflow (all matmuls contract over the SBUF partition dim, fp32
data with float32r single-pass matmuls; producers round to f32r on write):
  - PE-transpose x (q,k,v inputs) into xT [d, tok]  (exact fp32 transposes)
  - QT[dout, tok] = Wq.T chunks @ xT    (bias via K=1 ones-matmul)
  - KT[dout, tok] likewise; V[tok, dout] = xT.T chunks @ Wv (natural layout),
    stored head-interleaved with an appended ones column: [tok, h, 64+1]
  - scores transposed: sT[k, q] = KT_h.T @ QT_h  (K=dh=64), causal-trimmed
    at 128-col granularity (min matmul N=256 to stay on the f32r fast path)
  - P = exp(sT/8) on ACT; triangular mask multiply on diagonal blocks (DVE)
  - outT[65, q] += [V_h | 1].T @ P  accumulated over k chunks; row 64 = softmax
    denominators (via the ones column) -- no separate reduction needed
  - normalize: r = 1/sums (DVE approx reciprocal, batched over 8 heads at
    partitions {0,32,64}); broadcast r over 64 partitions via K=1 bf16
    matmul; DVE multiply into the staging buffer stage[dk, q]
  - out-proj: O[q, dout] = stage_chunks.T @ Wo chunks + bo (K=1 ones-matmul),
    already in natural layout for the output DMA.

repeat>1 wraps the whole per-batch computation in a hardware For_i loop --
used only by the timing harness to amortize the ~56 ms axon dispatch
overhead out of the per-iteration measurement.
"""

import numpy as np

B = 8
L = 2048
D = 512
H = 8
DH = 64
NT = L // 128   # 16 token tiles
NCH = D // 128  # 4 dim chunks
NQ = L // 512   # 4 q supertiles

_cached = {}


def _build(repeat=1):
    import concourse.bass as bass
    import concourse.tile as tile
    from concourse import mybir, bacc
    from concourse.masks import make_identity

    f32 = mybir.dt.float32
    bf16 = mybir.dt.bfloat16
    f32r = mybir.dt.float32r

    nc = bacc.Bacc("TRN2", target_bir_lowering=False, debug=False)

    xq = nc.dram_tensor("query", [L, D], f32, kind="ExternalInput").ap()
    xk = nc.dram_tensor("key", [L, D], f32, kind="ExternalInput").ap()
    xv = nc.dram_tensor("value", [L, D], f32, kind="ExternalInput").ap()
    Wq = nc.dram_tensor("Wq", [D, D], f32, kind="ExternalInput").ap()
    Wk = nc.dram_tensor("Wk", [D, D], f32, kind="ExternalInput").ap()
    Wv = nc.dram_tensor("Wv", [D, D], f32, kind="ExternalInput").ap()
    Wo = nc.dram_tensor("Wo", [D, D], f32, kind="ExternalInput").ap()
    bq = nc.dram_tensor("bq", [D], f32, kind="ExternalInput").ap()
    bk = nc.dram_tensor("bk", [D], f32, kind="ExternalInput").ap()
    bv = nc.dram_tensor("bv", [D], f32, kind="ExternalInput").ap()
    bo = nc.dram_tensor("bo", [D], f32, kind="ExternalInput").ap()
    out = nc.dram_tensor("out", [L, D], f32, kind="ExternalOutput").ap()

    def r(ap):
        return ap.bitcast(f32r)

    with tile.TileContext(nc) as tc:
        with (
            tc.tile_pool(name="persist", bufs=1) as persist,
            tc.tile_pool(name="consts", bufs=1) as consts,
        ):
            # ---- constants ----
            ident = consts.tile([128, 128], f32, tag="ident")
            make_identity(nc, ident[:])
            # tri[x, y] = 1.0 where y >= x else 0  (valid = q_local >= k_local)
            tri = consts.tile([128, 128], f32, tag="tri")
            nc.gpsimd.memset(tri[:], 0.0)
            nc.gpsimd.affine_select(
                out=tri[:], in_=tri[:], compare_op=mybir.AluOpType.is_gt,
                fill=1.0, base=0, pattern=[[-1, 128]], channel_multiplier=1,
            )
            # tri_wide[x, y] (y in [0,256)): 1.0 where y >= x + 128 else 0
            tri_wide = consts.tile([128, 256], f32, tag="tri_wide")
            nc.gpsimd.memset(tri_wide[:], 0.0)
            nc.gpsimd.affine_select(
                out=tri_wide[:], in_=tri_wide[:], compare_op=mybir.AluOpType.is_gt,
                fill=1.0, base=128, pattern=[[-1, 256]], channel_multiplier=1,
            )
            ones = consts.tile([1, 512], f32, tag="ones")
            nc.vector.memset(ones[:], 1.0)
            ones_t = consts.tile([128, 64], bf16, tag="ones_t")
            nc.vector.memset(ones_t[:], 1.0)

            # ---- weights / biases (DMA to temp, round-copy to f32r) ----
            w_sb = {}
            b_sb = {}
            with tc.tile_pool(name="wtmp", bufs=3) as wtmp_pool:
                for name, wdram in (("q", Wq), ("k", Wk), ("v", Wv), ("o", Wo)):
                    t = persist.tile([128, NCH, 512], f32, tag=f"W{name}",
                                     name=f"W{name}")
                    for c in range(NCH):
                        wt = wtmp_pool.tile([128, 512], f32, tag="wtmp",
                                            name="wtmp")
                        nc.sync.dma_start(wt[:], wdram[128 * c:128 * (c + 1), :])
                        nc.vector.tensor_copy(r(t[:, c, :]), wt[:])
                    w_sb[name] = t
                for name, bdram in (("q", bq), ("k", bk), ("v", bv), ("o", bo)):
                    t = persist.tile([1, 512], f32, tag=f"b{name}", name=f"b{name}")
                    bt = wtmp_pool.tile([1, 512], f32, tag="btmp", name="btmp",
                                        bufs=2)
                    nc.sync.dma_start(bt[:], bdram[None, :])
                    nc.vector.tensor_copy(r(t[:]), bt[:])
                    b_sb[name] = t

            # ---- persistent activations ----
            qt_sb = [persist.tile([128, L], f32, tag=f"QT{c}", name=f"QT{c}")
                     for c in range(NCH)]
            kt_sb = [persist.tile([128, L], f32, tag=f"KT{c}", name=f"KT{c}")
                     for c in range(NCH)]
            v_sb = [persist.tile([128, H, DH + 1], f32, tag=f"V{t}", name=f"V{t}")
                    for t in range(NT)]
            stage = [persist.tile([128, L], f32, tag=f"stage{c}", name=f"stage{c}")
                     for c in range(NCH)]

            def emit_phase_a():
                with (
                    tc.tile_pool(name="xin", bufs=6) as xin_pool,
                    tc.tile_pool(name="trps", bufs=3, space="PSUM") as trps_pool,
                    tc.tile_pool(name="xt", bufs=8) as xt_pool,
                    tc.tile_pool(name="projps", bufs=3, space="PSUM") as projps_pool,
                ):
                    for tname, xdram in (("q", xq), ("k", xk), ("v", xv)):
                        for g in range(NT // 4):  # groups of 4 token tiles
                            xtiles = []
                            for j in range(4):
                                t0 = 4 * g + j
                                xt_in = xin_pool.tile([128, 512], f32, tag="xin",
                                                      name="xin")
                                nc.sync.dma_start(
                                    xt_in[:], xdram[128 * t0:128 * (t0 + 1), :])
                                xtiles.append(xt_in)
                            # transpose to xT [dchunk][128 d, 512 toks]
                            xt_c = []
                            for c in range(NCH):
                                ps = trps_pool.tile([128, 512], f32, tag="trps",
                                                    name="trps")
                                for j in range(4):
                                    nc.tensor.transpose(
                                        ps[:, 128 * j:128 * (j + 1)],
                                        xtiles[j][:, 128 * c:128 * (c + 1)],
                                        ident[:],
                                    )
                                sb = xt_pool.tile([128, 512], f32, tag="xt",
                                                  name="xt")
                                nc.vector.tensor_copy(r(sb[:]), ps[:])
                                xt_c.append(sb)
                            if tname in ("q", "k"):
                                dst = qt_sb if tname == "q" else kt_sb
                                for co in range(NCH):
                                    pp = projps_pool.tile([128, 512], f32,
                                                          tag="projps",
                                                          name="projps")
                                    for ci in range(NCH):
                                        nc.tensor.matmul(
                                            pp[:],
                                            r(w_sb[tname][
                                                :, ci, 128 * co:128 * (co + 1)]),
                                            r(xt_c[ci][:]),
                                            start=(ci == 0), stop=False,
                                        )
                                    nc.tensor.matmul(
                                        pp[:],
                                        r(b_sb[tname][0:1, 128 * co:128 * (co + 1)]),
                                        r(ones[0:1, :]),
                                        start=False, stop=True,
                                    )
                                    nc.vector.tensor_copy(
                                        r(dst[co][:, 512 * g:512 * (g + 1)]), pp[:])
                            else:  # v: natural layout per token tile
                                for j in range(4):
                                    t0 = 4 * g + j
                                    pv = projps_pool.tile([128, 512], f32,
                                                          tag="projps",
                                                          name="projps")
                                    for ci in range(NCH):
                                        nc.tensor.matmul(
                                            pv[:],
                                            r(xt_c[ci][:, 128 * j:128 * (j + 1)]),
                                            r(w_sb["v"][:, ci, :]),
                                            start=(ci == 0), stop=False,
                                        )
                                    nc.tensor.matmul(
                                        pv[:], r(ones[0:1, 0:128]),
                                        r(b_sb["v"][0:1, :]),
                                        start=False, stop=True,
                                    )
                                    nc.vector.tensor_copy(
                                        r(v_sb[t0][:, :, 0:DH]),
                                        pv[:].rearrange("p (h d) -> p h d", h=H),
                                    )
                                    nc.gpsimd.memset(
                                        v_sb[t0][:, :, DH:DH + 1], 1.0)

            def emit_phase_bcd():
                # qt-outer so each q supertile finishes all heads, normalizes,
                # and projects while later supertiles still compute.
                with (
                    tc.tile_pool(name="sps", bufs=3, space="PSUM") as sps_pool,
                    tc.tile_pool(name="ops", bufs=2, space="PSUM") as ops_pool,
                    tc.tile_pool(name="bcps", bufs=1, space="PSUM") as bc_pool,
                    tc.tile_pool(name="outps", bufs=2, space="PSUM") as outps_pool,
                    tc.tile_pool(name="pexp", bufs=4) as p_pool,
                    tc.tile_pool(name="norm", bufs=1) as norm_pool,
                    tc.tile_pool(name="osb", bufs=3) as o_pool,
                ):
                    for qt in range(NQ):
                        kmax = 4 * qt + 4
                        # sums rows for the 8 heads at 32-aligned partitions:
                        # head h -> partition 32*(h%3), free offset 512*(h//3)
                        stg = norm_pool.tile([128, 1536], f32, tag="stg",
                                             name="stg")
                        for h in range(H):
                            ch, prow = h // 2, 64 * (h % 2)
                            kth = kt_sb[ch]
                            qth = qt_sb[ch]
                            po = ops_pool.tile([65, 512], f32, tag="ops",
                                               name="ops")
                            for c in range(kmax):
                                m = c - 4 * qt
                                # j0: first valid q col (128-gran), widened so
                                # matmul N >= 256 stays on the f32r fast path
                                j0 = 0 if m < 1 else (128 * m if m < 3 else 256)
                                ps = sps_pool.tile([128, 512], f32, tag="sps",
                                                   name="sps")
                                nc.tensor.matmul(
                                    ps[:, j0:512],
                                    r(kth[prow:prow + DH, 128 * c:128 * (c + 1)]),
                                    r(qth[prow:prow + DH,
                                          512 * qt + j0:512 * (qt + 1)]),
                                    start=True, stop=True,
                                )
                                pt = p_pool.tile([128, 512], f32, tag="pexp",
                                                 name="pexp")
                                nc.scalar.activation(
                                    r(pt[:, j0:512]), ps[:, j0:512],
                                    mybir.ActivationFunctionType.Exp, scale=0.125,
                                )
                                if m == 3:
                                    nc.vector.tensor_mul(
                                        r(pt[:, 256:512]), pt[:, 256:512],
                                        tri_wide[:])
                                elif m >= 0:
                                    nc.vector.tensor_mul(
                                        r(pt[:, 128 * m:128 * (m + 1)]),
                                        pt[:, 128 * m:128 * (m + 1)], tri[:])
                                nc.tensor.matmul(
                                    po[:, j0:512],
                                    r(v_sb[c][:, h, :]),
                                    r(pt[:, j0:512]),
                                    start=(c == 0), stop=(c == kmax - 1),
                                )
                            nc.vector.tensor_copy(
                                r(stage[ch][prow:prow + DH,
                                            512 * qt:512 * (qt + 1)]),
                                po[0:DH, :])
                            nc.vector.tensor_copy(
                                stg[32 * (h % 3):32 * (h % 3) + 1,
                                    512 * (h // 3):512 * (h // 3) + 512],
                                po[DH:DH + 1, :])
                        # reciprocal of all 8 sums rows (unused partitions hold
                        # garbage that is never read)
                        rstg = norm_pool.tile([128, 1536], f32, tag="rstg",
                                              name="rstg")
                        rscr = norm_pool.tile([128, 1536], f32, tag="rscr",
                                              name="rscr")
                        nc.vector.reciprocal_approx_accurate(
                            out=rstg[:], in_=stg[:], scratch=rscr[:])
                        rbf = norm_pool.tile([128, 1536], bf16, tag="rbf",
                                             name="rbf")
                        nc.vector.tensor_copy(rbf[:], rstg[:])
                        for ch in range(NCH):
                            bc = bc_pool.tile([128, 512], f32, tag="bcps",
                                              name="bcps")
                            for sub in range(2):
                                hh = 2 * ch + sub
                                pp0 = 32 * (hh % 3)
                                fo = 512 * (hh // 3)
                                nc.tensor.matmul(
                                    bc[64 * sub:64 * sub + 64, :],
                                    ones_t[pp0:pp0 + 1, 0:64],
                                    rbf[pp0:pp0 + 1, fo:fo + 512],
                                    start=True, stop=True,
                                )
                            nc.vector.tensor_mul(
                                r(stage[ch][:, 512 * qt:512 * (qt + 1)]),
                                stage[ch][:, 512 * qt:512 * (qt + 1)],
                                bc[:],
                            )
                        for i in range(4 * qt, 4 * qt + 4):
                            pout = outps_pool.tile([128, 512], f32, tag="outps",
                                                   name="outps")
                            for ch in range(NCH):
                                nc.tensor.matmul(
                                    pout[:],
                                    r(stage[ch][:, 128 * i:128 * (i + 1)]),
                                    r(w_sb["o"][:, ch, :]),
                                    start=(ch == 0), stop=False,
                                )
                            nc.tensor.matmul(
                                pout[:], r(ones[0:1, 0:128]), r(b_sb["o"][0:1, :]),
                                start=False, stop=True,
                            )
                            ot = o_pool.tile([128, 512], f32, tag="osb",
                                             name="osb")
                            nc.vector.tensor_copy(ot[:], pout[:])
                            nc.sync.dma_start(
                                out[128 * i:128 * (i + 1), :], ot[:])

            def emit_body():
                emit_phase_a()
                emit_phase_bcd()

            if repeat > 1:
                with tc.For_i(0, repeat, 1, hint_engines=(
                        mybir.EngineType.PE,
                        mybir.EngineType.DVE,
                        mybir.EngineType.Activation,
                        mybir.EngineType.SP,
                        mybir.EngineType.Pool)):
                    emit_body()
            else:
                emit_body()

    nc.compile()
    return nc


def get_nc(repeat=1):
    key = f"nc{repeat}"
    if key not in _cached:
        _cached[key] = _build(repeat)
    return _cached[key]


def run(in_maps, trace=False, repeat=1, **kw):
    from concourse.bass_utils import run_bass_kernel_spmd

    nc = get_nc(repeat)
    return run_bass_kernel_spmd(nc, in_maps, list(range(B)), trace=trace, **kw)


def kernel(query, key, value, Wq, bq, Wk, bk, Wv, bv, Wo, bo):
    shared = {
        "Wq": np.ascontiguousarray(Wq, np.float32),
        "Wk": np.ascontiguousarray(Wk, np.float32),
        "Wv": np.ascontiguousarray(Wv, np.float32),
        "Wo": np.ascontiguousarray(Wo, np.float32),
        "bq": np.ascontiguousarray(bq, np.float32),
        "bk": np.ascontiguousarray(bk, np.float32),
        "bv": np.ascontiguousarray(bv, np.float32),
        "bo": np.ascontiguousarray(bo, np.float32),
    }
    in_maps = []
    for i in range(B):
        m = dict(shared)
        m["query"] = np.ascontiguousarray(query[i], np.float32)
        m["key"] = np.ascontiguousarray(key[i], np.float32)
        m["value"] = np.ascontiguousarray(value[i], np.float32)
        in_maps.append(m)
    res = run(in_maps)
    return np.stack([res.results[i]["out"] for i in range(B)], axis=0)


# revision 15
# speedup vs baseline: 1.5884x; 1.1533x over previous
"""Causal multi-head attention layer for Trainium2, 8-core data-parallel.

Problem: B=8, L=2048, D=512, H=8 heads (dh=64), fp32, causal softmax attention
with QKV/out projections + biases.

Sharding: pure data parallel over batch (B=8 == n_cores); each core runs one
batch element end-to-end; no collectives.

Per-core dataflow (all matmuls contract over the SBUF partition dim, fp32
data with float32r single-pass matmuls; producers round to f32r on write):
  - PE-transpose x (q,k,v inputs) into xT [d, tok]  (exact fp32 transposes)
  - QT[dout, tok] = Wq.T chunks @ xT    (bias via K=1 ones-matmul)
  - KT[dout, tok] likewise; V[tok, dout] = xT.T chunks @ Wv (natural layout),
    stored head-interleaved with an appended ones column: [tok, h, 64+1]
  - scores transposed: sT[k, q] = KT_h.T @ QT_h  (K=dh=64), causal-trimmed
    at 128-col granularity (min matmul N=256 to stay on the f32r fast path)
  - P = exp(sT/8) on ACT; triangular mask multiply on diagonal blocks (DVE)
  - outT[65, q] += [V_h | 1].T @ P  accumulated over k chunks; row 64 = softmax
    denominators (via the ones column) -- no separate reduction needed
  - normalize: r = 1/sums (DVE approx reciprocal, batched over 8 heads at
    partitions {0,32,64}); broadcast r over 64 partitions via K=1 bf16
    matmul; DVE multiply into the staging buffer stage[dk, q]
  - out-proj: O[q, dout] = stage_chunks.T @ Wo chunks + bo (K=1 ones-matmul),
    already in natural layout for the output DMA.

repeat>1 wraps the whole per-batch computation in a hardware For_i loop --
used only by the timing harness to amortize the ~56 ms axon dispatch
overhead out of the per-iteration measurement.
"""

import numpy as np

B = 8
L = 2048
D = 512
H = 8
DH = 64
NT = L // 128   # 16 token tiles
NCH = D // 128  # 4 dim chunks
NQ = L // 512   # 4 q supertiles

_cached = {}


def _build(repeat=1, phases="full", cfg=None):
    cfg = dict(cfg or {})
    SPS = cfg.get("sps", 3); OPS = cfg.get("ops", 2); PEXP = cfg.get("pexp", 4)
    OUTPS = cfg.get("outps", 2); BCPS = cfg.get("bcps", 1); OSB = cfg.get("osb", 3)
    XIN = cfg.get("xin", 6); TRPS = cfg.get("trps", 3); XT = cfg.get("xt", 8)
    PROJPS = cfg.get("projps", 3)
    import concourse.bass as bass
    import concourse.tile as tile
    from concourse import mybir, bacc
    from concourse.masks import make_identity

    f32 = mybir.dt.float32
    bf16 = mybir.dt.bfloat16
    f32r = mybir.dt.float32r

    nc = bacc.Bacc("TRN2", target_bir_lowering=False, debug=False)

    xq = nc.dram_tensor("query", [L, D], f32, kind="ExternalInput").ap()
    xk = nc.dram_tensor("key", [L, D], f32, kind="ExternalInput").ap()
    xv = nc.dram_tensor("value", [L, D], f32, kind="ExternalInput").ap()
    Wq = nc.dram_tensor("Wq", [D, D], f32, kind="ExternalInput").ap()
    Wk = nc.dram_tensor("Wk", [D, D], f32, kind="ExternalInput").ap()
    Wv = nc.dram_tensor("Wv", [D, D], f32, kind="ExternalInput").ap()
    Wo = nc.dram_tensor("Wo", [D, D], f32, kind="ExternalInput").ap()
    bq = nc.dram_tensor("bq", [D], f32, kind="ExternalInput").ap()
    bk = nc.dram_tensor("bk", [D], f32, kind="ExternalInput").ap()
    bv = nc.dram_tensor("bv", [D], f32, kind="ExternalInput").ap()
    bo = nc.dram_tensor("bo", [D], f32, kind="ExternalInput").ap()
    out = nc.dram_tensor("out", [L, D], f32, kind="ExternalOutput").ap()

    def r(ap):
        return ap.bitcast(f32r)

    with tile.TileContext(nc) as tc:
        with (
            tc.tile_pool(name="persist", bufs=1) as persist,
            tc.tile_pool(name="consts", bufs=1) as consts,
        ):
            # ---- constants ----
            ident = consts.tile([128, 128], f32, tag="ident")
            make_identity(nc, ident[:])
            # tri[x, y] = 1.0 where y >= x else 0  (valid = q_local >= k_local)
            tri = consts.tile([128, 128], f32, tag="tri")
            nc.gpsimd.memset(tri[:], 0.0)
            nc.gpsimd.affine_select(
                out=tri[:], in_=tri[:], compare_op=mybir.AluOpType.is_gt,
                fill=1.0, base=0, pattern=[[-1, 128]], channel_multiplier=1,
            )
            # tri_wide[x, y] (y in [0,256)): 1.0 where y >= x + 128 else 0
            tri_wide = consts.tile([128, 256], f32, tag="tri_wide")
            nc.gpsimd.memset(tri_wide[:], 0.0)
            nc.gpsimd.affine_select(
                out=tri_wide[:], in_=tri_wide[:], compare_op=mybir.AluOpType.is_gt,
                fill=1.0, base=128, pattern=[[-1, 256]], channel_multiplier=1,
            )
            ones = consts.tile([1, 512], f32, tag="ones")
            nc.vector.memset(ones[:], 1.0)
            ones_t = consts.tile([128, 64], bf16, tag="ones_t")
            nc.vector.memset(ones_t[:], 1.0)

            # ---- weights / biases (DMA to temp, round-copy to f32r) ----
            w_sb = {}
            b_sb = {}
            with tc.tile_pool(name="wtmp", bufs=3) as wtmp_pool:
                for name, wdram in (("q", Wq), ("k", Wk), ("v", Wv), ("o", Wo)):
                    t = persist.tile([128, NCH, 512], f32, tag=f"W{name}",
                                     name=f"W{name}")
                    for c in range(NCH):
                        wt = wtmp_pool.tile([128, 512], f32, tag="wtmp",
                                            name="wtmp")
                        nc.sync.dma_start(wt[:], wdram[128 * c:128 * (c + 1), :])
                        nc.vector.tensor_copy(r(t[:, c, :]), wt[:])
                    w_sb[name] = t
                for name, bdram in (("q", bq), ("k", bk), ("v", bv), ("o", bo)):
                    t = persist.tile([1, 512], f32, tag=f"b{name}", name=f"b{name}")
                    bt = wtmp_pool.tile([1, 512], f32, tag="btmp", name="btmp",
                                        bufs=2)
                    nc.sync.dma_start(bt[:], bdram[None, :])
                    nc.vector.tensor_copy(r(t[:]), bt[:])
                    b_sb[name] = t

            # ---- persistent activations ----
            qt_sb = [persist.tile([128, L], f32, tag=f"QT{c}", name=f"QT{c}")
                     for c in range(NCH)]
            kt_sb = [persist.tile([128, L], f32, tag=f"KT{c}", name=f"KT{c}")
                     for c in range(NCH)]
            v_sb = [persist.tile([128, H, DH + 1], f32, tag=f"V{t}", name=f"V{t}")
                    for t in range(NT)]
            stage = [persist.tile([128, L], f32, tag=f"stage{c}", name=f"stage{c}")
                     for c in range(NCH)]

            def emit_phase_a():
                with (
                    tc.tile_pool(name="xin", bufs=XIN) as xin_pool,
                    tc.tile_pool(name="trps", bufs=TRPS, space="PSUM") as trps_pool,
                    tc.tile_pool(name="xt", bufs=XT) as xt_pool,
                    tc.tile_pool(name="projps", bufs=PROJPS, space="PSUM") as projps_pool,
                ):
                    for tname, xdram in (("q", xq), ("k", xk), ("v", xv)):
                        for g in range(NT // 4):  # groups of 4 token tiles
                            xtiles = []
                            for j in range(4):
                                t0 = 4 * g + j
                                xt_in = xin_pool.tile([128, 512], f32, tag="xin",
                                                      name="xin")
                                nc.sync.dma_start(
                                    xt_in[:], xdram[128 * t0:128 * (t0 + 1), :])
                                xtiles.append(xt_in)
                            # transpose to xT [dchunk][128 d, 512 toks]
                            xt_c = []
                            for c in range(NCH):
                                ps = trps_pool.tile([128, 512], f32, tag="trps",
                                                    name="trps")
                                for j in range(4):
                                    nc.tensor.transpose(
                                        ps[:, 128 * j:128 * (j + 1)],
                                        xtiles[j][:, 128 * c:128 * (c + 1)],
                                        ident[:],
                                    )
                                sb = xt_pool.tile([128, 512], f32, tag="xt",
                                                  name="xt")
                                nc.vector.tensor_copy(r(sb[:]), ps[:])
                                xt_c.append(sb)
                            if tname in ("q", "k"):
                                dst = qt_sb if tname == "q" else kt_sb
                                for co in range(NCH):
                                    pp = projps_pool.tile([128, 512], f32,
                                                          tag="projps",
                                                          name="projps")
                                    for ci in range(NCH):
                                        nc.tensor.matmul(
                                            pp[:],
                                            r(w_sb[tname][
                                                :, ci, 128 * co:128 * (co + 1)]),
                                            r(xt_c[ci][:]),
                                            start=(ci == 0), stop=False,
                                        )
                                    nc.tensor.matmul(
                                        pp[:],
                                        r(b_sb[tname][0:1, 128 * co:128 * (co + 1)]),
                                        r(ones[0:1, :]),
                                        start=False, stop=True,
                                    )
                                    nc.vector.tensor_copy(
                                        r(dst[co][:, 512 * g:512 * (g + 1)]), pp[:])
                            else:  # v: natural layout per token tile
                                for j in range(4):
                                    t0 = 4 * g + j
                                    pv = projps_pool.tile([128, 512], f32,
                                                          tag="projps",
                                                          name="projps")
                                    for ci in range(NCH):
                                        nc.tensor.matmul(
                                            pv[:],
                                            r(xt_c[ci][:, 128 * j:128 * (j + 1)]),
                                            r(w_sb["v"][:, ci, :]),
                                            start=(ci == 0), stop=False,
                                        )
                                    nc.tensor.matmul(
                                        pv[:], r(ones[0:1, 0:128]),
                                        r(b_sb["v"][0:1, :]),
                                        start=False, stop=True,
                                    )
                                    nc.vector.tensor_copy(
                                        r(v_sb[t0][:, :, 0:DH]),
                                        pv[:].rearrange("p (h d) -> p h d", h=H),
                                    )
                                    nc.gpsimd.memset(
                                        v_sb[t0][:, :, DH:DH + 1], 1.0)

            def emit_phase_bcd():
                # qt-outer so each q supertile finishes all heads, normalizes,
                # and projects while later supertiles still compute.
                with (
                    tc.tile_pool(name="sps", bufs=SPS, space="PSUM") as sps_pool,
                    tc.tile_pool(name="ops", bufs=OPS, space="PSUM") as ops_pool,
                    tc.tile_pool(name="bcps", bufs=BCPS, space="PSUM") as bc_pool,
                    tc.tile_pool(name="outps", bufs=OUTPS, space="PSUM") as outps_pool,
                    tc.tile_pool(name="pexp", bufs=PEXP) as p_pool,
                    tc.tile_pool(name="norm", bufs=1) as norm_pool,
                    tc.tile_pool(name="osb", bufs=OSB) as o_pool,
                ):
                    for qt in range(NQ):
                        kmax = 4 * qt + 4
                        # sums rows for the 8 heads at 32-aligned partitions:
                        # head h -> partition 32*(h%3), free offset 512*(h//3)
                        stg = norm_pool.tile([128, 1536], f32, tag="stg",
                                             name="stg")
                        for h in range(H):
                            ch, prow = h // 2, 64 * (h % 2)
                            kth = kt_sb[ch]
                            qth = qt_sb[ch]
                            po = ops_pool.tile([65, 512], f32, tag="ops",
                                               name="ops")
                            for c in range(kmax):
                                m = c - 4 * qt
                                # j0: first valid q col (128-gran), widened so
                                # matmul N >= 256 stays on the f32r fast path
                                j0 = 0 if m < 1 else (128 * m if m < 3 else 256)
                                ps = sps_pool.tile([128, 512], f32, tag="sps",
                                                   name="sps")
                                nc.tensor.matmul(
                                    ps[:, j0:512],
                                    r(kth[prow:prow + DH, 128 * c:128 * (c + 1)]),
                                    r(qth[prow:prow + DH,
                                          512 * qt + j0:512 * (qt + 1)]),
                                    start=True, stop=True,
                                )
                                pt = p_pool.tile([128, 512], f32, tag="pexp",
                                                 name="pexp")
                                nc.scalar.activation(
                                    r(pt[:, j0:512]), ps[:, j0:512],
                                    mybir.ActivationFunctionType.Exp, scale=0.125,
                                )
                                if m == 3:
                                    nc.vector.tensor_mul(
                                        r(pt[:, 256:512]), pt[:, 256:512],
                                        tri_wide[:])
                                elif m >= 0:
                                    nc.vector.tensor_mul(
                                        r(pt[:, 128 * m:128 * (m + 1)]),
                                        pt[:, 128 * m:128 * (m + 1)], tri[:])
                                nc.tensor.matmul(
                                    po[:, j0:512],
                                    r(v_sb[c][:, h, :]),
                                    r(pt[:, j0:512]),
                                    start=(c == 0), stop=(c == kmax - 1),
                                )
                            nc.vector.tensor_copy(
                                r(stage[ch][prow:prow + DH,
                                            512 * qt:512 * (qt + 1)]),
                                po[0:DH, :])
                            nc.vector.tensor_copy(
                                stg[32 * (h % 3):32 * (h % 3) + 1,
                                    512 * (h // 3):512 * (h // 3) + 512],
                                po[DH:DH + 1, :])
                        # reciprocal of all 8 sums rows (unused partitions hold
                        # garbage that is never read)
                        rstg = norm_pool.tile([128, 1536], f32, tag="rstg",
                                              name="rstg")
                        rscr = norm_pool.tile([128, 1536], f32, tag="rscr",
                                              name="rscr")
                        nc.vector.reciprocal_approx_accurate(
                            out=rstg[:], in_=stg[:], scratch=rscr[:])
                        rbf = norm_pool.tile([128, 1536], bf16, tag="rbf",
                                             name="rbf")
                        nc.vector.tensor_copy(rbf[:], rstg[:])
                        for ch in range(NCH):
                            bc = bc_pool.tile([128, 512], f32, tag="bcps",
                                              name="bcps")
                            for sub in range(2):
                                hh = 2 * ch + sub
                                pp0 = 32 * (hh % 3)
                                fo = 512 * (hh // 3)
                                nc.tensor.matmul(
                                    bc[64 * sub:64 * sub + 64, :],
                                    ones_t[pp0:pp0 + 1, 0:64],
                                    rbf[pp0:pp0 + 1, fo:fo + 512],
                                    start=True, stop=True,
                                )
                            nc.vector.tensor_mul(
                                r(stage[ch][:, 512 * qt:512 * (qt + 1)]),
                                stage[ch][:, 512 * qt:512 * (qt + 1)],
                                bc[:],
                            )
                        for i in range(4 * qt, 4 * qt + 4):
                            pout = outps_pool.tile([128, 512], f32, tag="outps",
                                                   name="outps")
                            for ch in range(NCH):
                                nc.tensor.matmul(
                                    pout[:],
                                    r(stage[ch][:, 128 * i:128 * (i + 1)]),
                                    r(w_sb["o"][:, ch, :]),
                                    start=(ch == 0), stop=False,
                                )
                            nc.tensor.matmul(
                                pout[:], r(ones[0:1, 0:128]), r(b_sb["o"][0:1, :]),
                                start=False, stop=True,
                            )
                            ot = o_pool.tile([128, 512], f32, tag="osb",
                                             name="osb")
                            nc.vector.tensor_copy(ot[:], pout[:])
                            nc.sync.dma_start(
                                out[128 * i:128 * (i + 1), :], ot[:])

            def emit_body():
                if "a" in phases or phases == "full":
                    emit_phase_a()
                if phases == "full" or "b" in phases:
                    emit_phase_bcd()

            if repeat > 1:
                with tc.For_i(0, repeat, 1, hint_engines=(
                        mybir.EngineType.PE,
                        mybir.EngineType.DVE,
                        mybir.EngineType.Activation,
                        mybir.EngineType.SP,
                        mybir.EngineType.Pool)):
                    emit_body()
            else:
                emit_body()

    nc.compile()
    return nc


def get_nc(repeat=1, phases="full", cfg=None):
    key = f"nc{repeat}-{phases}-{sorted((cfg or {}).items())}"
    if key not in _cached:
        _cached[key] = _build(repeat, phases, cfg)
    return _cached[key]


def run(in_maps, trace=False, repeat=1, **kw):
    from concourse.bass_utils import run_bass_kernel_spmd

    nc = get_nc(repeat)
    return run_bass_kernel_spmd(nc, in_maps, list(range(B)), trace=trace, **kw)


def kernel(query, key, value, Wq, bq, Wk, bk, Wv, bv, Wo, bo):
    shared = {
        "Wq": np.ascontiguousarray(Wq, np.float32),
        "Wk": np.ascontiguousarray(Wk, np.float32),
        "Wv": np.ascontiguousarray(Wv, np.float32),
        "Wo": np.ascontiguousarray(Wo, np.float32),
        "bq": np.ascontiguousarray(bq, np.float32),
        "bk": np.ascontiguousarray(bk, np.float32),
        "bv": np.ascontiguousarray(bv, np.float32),
        "bo": np.ascontiguousarray(bo, np.float32),
    }
    in_maps = []
    for i in range(B):
        m = dict(shared)
        m["query"] = np.ascontiguousarray(query[i], np.float32)
        m["key"] = np.ascontiguousarray(key[i], np.float32)
        m["value"] = np.ascontiguousarray(value[i], np.float32)
        in_maps.append(m)
    res = run(in_maps)
    return np.stack([res.results[i]["out"] for i in range(B)], axis=0)


# revision 17
# speedup vs baseline: 1.7814x; 1.1215x over previous
"""v3: fine-grained weave of projections into attention heads.

Differences vs v1:
  - Phase A (transpose+projections) is emitted per token-group g, immediately
    followed by the full attention/normalize/out-proj for q supertile qt=g
    (which only depends on groups <= g). PE-heavy A work fills the PE idle
    of the ACT-bound attention phase.
  - One shared [128,512] PSUM pool (bufs=6) for transposes/projections/
    scores/bcast/out-proj + a dedicated [65,512] accumulation pool (bufs=2).
  - All biases are folded into DVE ops instead of K=1 matmuls:
      QT/KT: tensor_scalar_add with per-partition bias columns (PE-transposed
      from the bias rows once at startup)
      V/O: tensor_tensor add with pre-broadcast bias tiles (built once via a
      single ones-matmul each)
"""

import numpy as np

B = 8
L = 2048
D = 512
H = 8
DH = 64
NT = L // 128
NCH = D // 128
NQ = L // 512

_cached = {}


def _build(repeat=1, cfg=None):
    cfg = dict(cfg or {})
    PS512 = cfg.get("ps512", 3)
    SPS = cfg.get("sps", 3)
    OPS = cfg.get("ops", 2)
    PEXP = cfg.get("pexp", 4)
    OSB = cfg.get("osb", 2)
    XIN = cfg.get("xin", 5)
    XT = cfg.get("xt", 6)
    import concourse.tile as tile
    from concourse import mybir, bacc
    from concourse.masks import make_identity

    f32 = mybir.dt.float32
    bf16 = mybir.dt.bfloat16
    f32r = mybir.dt.float32r

    nc = bacc.Bacc("TRN2", target_bir_lowering=False, debug=False)

    xq = nc.dram_tensor("query", [L, D], f32, kind="ExternalInput").ap()
    xk = nc.dram_tensor("key", [L, D], f32, kind="ExternalInput").ap()
    xv = nc.dram_tensor("value", [L, D], f32, kind="ExternalInput").ap()
    Wq = nc.dram_tensor("Wq", [D, D], f32, kind="ExternalInput").ap()
    Wk = nc.dram_tensor("Wk", [D, D], f32, kind="ExternalInput").ap()
    Wv = nc.dram_tensor("Wv", [D, D], f32, kind="ExternalInput").ap()
    Wo = nc.dram_tensor("Wo", [D, D], f32, kind="ExternalInput").ap()
    bq = nc.dram_tensor("bq", [D], f32, kind="ExternalInput").ap()
    bk = nc.dram_tensor("bk", [D], f32, kind="ExternalInput").ap()
    bv = nc.dram_tensor("bv", [D], f32, kind="ExternalInput").ap()
    bo = nc.dram_tensor("bo", [D], f32, kind="ExternalInput").ap()
    out = nc.dram_tensor("out", [L, D], f32, kind="ExternalOutput").ap()

    def r(ap):
        return ap.bitcast(f32r)

    with tile.TileContext(nc) as tc:
        with (
            tc.tile_pool(name="persist", bufs=1) as persist,
            tc.tile_pool(name="consts", bufs=1) as consts,
            tc.tile_pool(name="ps512", bufs=PS512, space="PSUM") as ps512,
            tc.tile_pool(name="sps", bufs=SPS, space="PSUM") as sps_pool,
            tc.tile_pool(name="ops", bufs=OPS, space="PSUM") as ops_pool,
        ):
            # ---- constants ----
            ident = consts.tile([128, 128], f32, tag="ident")
            make_identity(nc, ident[:])
            tri = consts.tile([128, 128], bf16, tag="tri")
            nc.gpsimd.memset(tri[:], 0.0)
            nc.gpsimd.affine_select(
                out=tri[:], in_=tri[:], compare_op=mybir.AluOpType.is_gt,
                fill=1.0, base=0, pattern=[[-1, 128]], channel_multiplier=1,
            )
            tri_wide = consts.tile([128, 256], bf16, tag="tri_wide")
            nc.gpsimd.memset(tri_wide[:], 0.0)
            nc.gpsimd.affine_select(
                out=tri_wide[:], in_=tri_wide[:], compare_op=mybir.AluOpType.is_gt,
                fill=1.0, base=128, pattern=[[-1, 256]], channel_multiplier=1,
            )
            ones = consts.tile([1, 512], f32, tag="ones")
            nc.vector.memset(ones[:], 1.0)
            ones_t = consts.tile([128, 64], bf16, tag="ones_t")
            nc.vector.memset(ones_t[:], 1.0)

            # ---- weights / biases ----
            w_sb = {}
            b_row = {}
            with tc.tile_pool(name="wtmp", bufs=3) as wtmp_pool:
                for name, wdram in (("q", Wq), ("k", Wk), ("v", Wv), ("o", Wo)):
                    t = persist.tile([128, NCH, 512], f32, tag=f"W{name}",
                                     name=f"W{name}")
                    for c in range(NCH):
                        wt = wtmp_pool.tile([128, 512], f32, tag="wtmp",
                                            name="wtmp")
                        nc.sync.dma_start(wt[:], wdram[128 * c:128 * (c + 1), :])
                        nc.vector.tensor_copy(r(t[:, c, :]), wt[:])
                    w_sb[name] = t
                for name, bdram in (("q", bq), ("k", bk), ("v", bv), ("o", bo)):
                    t = wtmp_pool.tile([1, 512], f32, tag=f"b{name}",
                                       name=f"b{name}", bufs=1)
                    nc.sync.dma_start(t[:], bdram[None, :])
                    b_row[name] = t
                # per-partition bias columns for q/k (dout on partitions)
                bcol = {}
                for name in ("q", "k"):
                    bc_t = consts.tile([128, NCH], f32, tag=f"bcol{name}",
                                       name=f"bcol{name}")
                    for c in range(NCH):
                        tp = ps512.tile([128, 512], f32, tag="ps512", name="ps512")
                        nc.tensor.transpose(
                            tp[:, 0:1], b_row[name][0:1, 128 * c:128 * (c + 1)],
                            ident[0:1, 0:1])
                        nc.vector.tensor_copy(bc_t[:, c:c + 1], tp[:, 0:1])
                    bcol[name] = bc_t
                # broadcast bias tiles for v (head-interleaved) and o (natural)
                bvb = consts.tile([128, H, DH], f32, tag="bvb", name="bvb")
                bob = consts.tile([128, 512], f32, tag="bob", name="bob")
                for dst, row in ((bvb, b_row["v"]), (bob, b_row["o"])):
                    rowr = wtmp_pool.tile([1, 512], f32, tag="browr",
                                          name="browr", bufs=2)
                    nc.vector.tensor_copy(r(rowr[:]), row[:])
                    tp = ps512.tile([128, 512], f32, tag="ps512", name="ps512")
                    nc.tensor.matmul(tp[:], r(ones[0:1, 0:128]), r(rowr[:]),
                                     start=True, stop=True)
                    if dst is bvb:
                        nc.vector.tensor_copy(
                            dst[:], tp[:].rearrange("p (h d) -> p h d", h=H))
                    else:
                        nc.vector.tensor_copy(dst[:], tp[:])

            # ---- persistent activations ----
            kt_sb = [persist.tile([128, L], f32, tag=f"KT{c}", name=f"KT{c}")
                     for c in range(NCH)]
            v_sb = [persist.tile([128, H, DH + 1], bf16, tag=f"V{t}",
                        name=f"V{t}") for t in range(NT)]
            stage = [persist.tile([128, L], f32, tag=f"stage{c}", name=f"stage{c}")
                     for c in range(NCH)]

            with (
                tc.tile_pool(name="xin", bufs=XIN) as xin_pool,
                tc.tile_pool(name="qtg", bufs=2) as qtg_pool,
                tc.tile_pool(name="xt", bufs=XT) as xt_pool,
                tc.tile_pool(name="pexp", bufs=PEXP) as p_pool,
                tc.tile_pool(name="norm", bufs=1) as norm_pool,
                tc.tile_pool(name="osb", bufs=OSB) as o_pool,
            ):
                def emit_a_pieces(g):
                    """Return (qt_g, [thunk, ...]) -- pieces of the
                    transpose+projection work for token group g, to be woven
                    between attention heads of the previous supertile."""
                    qt_g = [qtg_pool.tile([128, 512], f32, tag=f"qtg{c}",
                                          name=f"qtg{c}") for c in range(NCH)]
                    pieces = []
                    for tname_, xdram_ in (("k", xk), ("v", xv), ("q", xq)):
                        pieces.append(
                            lambda tname=tname_, xdram=xdram_: emit_a_tensor(
                                g, tname, xdram, qt_g))
                    return qt_g, pieces

                def emit_a_tensor(g, tname, xdram, qt_g):
                    if True:
                        xtiles = []
                        for j in range(4):
                            t0 = 4 * g + j
                            xt_in = xin_pool.tile([128, 512], f32, tag="xin",
                                                  name="xin")
                            nc.sync.dma_start(
                                xt_in[:], xdram[128 * t0:128 * (t0 + 1), :])
                            xtiles.append(xt_in)
                        xt_c = []
                        for c in range(NCH):
                            ps = ps512.tile([128, 512], f32, tag="ps512",
                                            name="ps512")
                            for j in range(4):
                                nc.tensor.transpose(
                                    ps[:, 128 * j:128 * (j + 1)],
                                    xtiles[j][:, 128 * c:128 * (c + 1)],
                                    ident[:],
                                )
                            sb = xt_pool.tile([128, 512], f32, tag="xt", name="xt")
                            nc.vector.tensor_copy(r(sb[:]), ps[:])
                            xt_c.append(sb)
                        if tname in ("q", "k"):
                            for co in range(NCH):
                                pp = ps512.tile([128, 512], f32, tag="ps512",
                                                name="ps512")
                                for ci in range(NCH):
                                    nc.tensor.matmul(
                                        pp[:],
                                        r(w_sb[tname][
                                            :, ci, 128 * co:128 * (co + 1)]),
                                        r(xt_c[ci][:]),
                                        start=(ci == 0), stop=(ci == NCH - 1),
                                    )
                                if tname == "q":
                                    nc.vector.tensor_scalar_add(
                                        r(qt_g[co][:]), pp[:],
                                        bcol["q"][:, co:co + 1])
                                else:
                                    nc.vector.tensor_scalar_add(
                                        r(kt_sb[co][:, 512 * g:512 * (g + 1)]),
                                        pp[:], bcol["k"][:, co:co + 1])
                        else:
                            for j in range(4):
                                t0 = 4 * g + j
                                pv = ps512.tile([128, 512], f32, tag="ps512",
                                                name="ps512")
                                for ci in range(NCH):
                                    nc.tensor.matmul(
                                        pv[:],
                                        r(xt_c[ci][:, 128 * j:128 * (j + 1)]),
                                        r(w_sb["v"][:, ci, :]),
                                        start=(ci == 0), stop=(ci == NCH - 1),
                                    )
                                nc.vector.tensor_add(
                                    v_sb[t0][:, :, 0:DH],
                                    pv[:].rearrange("p (h d) -> p h d", h=H),
                                    bvb[:],
                                )
                                nc.gpsimd.memset(v_sb[t0][:, :, DH:DH + 1], 1.0)

                def emit_b_qt(qt, qt_g, weave=()):
                    weave = list(weave)
                    """Attention + normalize + out-proj for q supertile qt."""
                    kmax = 4 * qt + 4
                    stg = norm_pool.tile([128, 1536], f32, tag="stg", name="stg")
                    for h in range(H):
                        ch, prow = h // 2, 64 * (h % 2)
                        kth = kt_sb[ch]
                        qth = qt_g[ch]
                        po = ops_pool.tile([65, 512], f32, tag="ops", name="ops")
                        for c in range(kmax):
                            m = c - 4 * qt
                            # scores (f32r) need matmul N >= 256; the bf16
                            # attn@V matmul has no such constraint
                            js0 = 0 if m < 1 else (128 * m if m < 3 else 256)
                            jv0 = 0 if m < 1 else 128 * m
                            ps = sps_pool.tile([128, 512], f32, tag="sps",
                                               name="sps")
                            nc.tensor.matmul(
                                ps[:, js0:512],
                                r(kth[prow:prow + DH, 128 * c:128 * (c + 1)]),
                                r(qth[prow:prow + DH, js0:512]),
                                start=True, stop=True,
                            )
                            pt = p_pool.tile([128, 512], bf16, tag="pexp",
                                             name="pexp")
                            nc.scalar.activation(
                                pt[:, jv0:512], ps[:, jv0:512],
                                mybir.ActivationFunctionType.Exp, scale=0.125,
                            )
                            if m >= 0:
                                nc.vector.tensor_mul(
                                    pt[:, 128 * m:128 * (m + 1)],
                                    pt[:, 128 * m:128 * (m + 1)], tri[:])
                            nc.tensor.matmul(
                                po[:, jv0:512],
                                v_sb[c][:, h, :],
                                pt[:, jv0:512],
                                start=(c == 0), stop=(c == kmax - 1),
                            )
                        nc.vector.tensor_copy(
                            r(stage[ch][prow:prow + DH,
                                        512 * qt:512 * (qt + 1)]),
                            po[0:DH, :])
                        nc.vector.tensor_copy(
                            stg[32 * (h % 3):32 * (h % 3) + 1,
                                512 * (h // 3):512 * (h // 3) + 512],
                            po[DH:DH + 1, :])
                        if weave:
                            weave.pop(0)()
                    rstg = norm_pool.tile([128, 1536], f32, tag="rstg",
                                          name="rstg")
                    nc.vector.reciprocal_approx_fast(out=rstg[:], in_=stg[:])
                    rbf = norm_pool.tile([128, 1536], bf16, tag="rbf", name="rbf")
                    nc.vector.tensor_copy(rbf[:], rstg[:])
                    for ch in range(NCH):
                        bcp = ps512.tile([128, 512], f32, tag="ps512",
                                         name="ps512")
                        for sub in range(2):
                            hh = 2 * ch + sub
                            pp0 = 32 * (hh % 3)
                            fo = 512 * (hh // 3)
                            nc.tensor.matmul(
                                bcp[64 * sub:64 * sub + 64, :],
                                ones_t[pp0:pp0 + 1, 0:64],
                                rbf[pp0:pp0 + 1, fo:fo + 512],
                                start=True, stop=True,
                            )
                        nc.vector.tensor_mul(
                            r(stage[ch][:, 512 * qt:512 * (qt + 1)]),
                            stage[ch][:, 512 * qt:512 * (qt + 1)],
                            bcp[:],
                        )
                    for i in range(4 * qt, 4 * qt + 4):
                        pout = ps512.tile([128, 512], f32, tag="ps512",
                                          name="ps512")
                        for ch in range(NCH):
                            nc.tensor.matmul(
                                pout[:],
                                r(stage[ch][:, 128 * i:128 * (i + 1)]),
                                r(w_sb["o"][:, ch, :]),
                                start=(ch == 0), stop=(ch == NCH - 1),
                            )
                        ot = o_pool.tile([128, 512], f32, tag="osb", name="osb")
                        nc.vector.tensor_add(ot[:], pout[:], bob[:])
                        nc.sync.dma_start(out[128 * i:128 * (i + 1), :], ot[:])
                    for w in weave:
                        w()

                def emit_body():
                    qt_g, pieces = emit_a_pieces(0)
                    for p in pieces:
                        p()
                    for g in range(NQ):
                        if g + 1 < NQ:
                            qt_next, weave = emit_a_pieces(g + 1)
                        else:
                            qt_next, weave = None, ()
                        emit_b_qt(g, qt_g, weave)
                        qt_g = qt_next

                if repeat > 1:
                    with tc.For_i(0, repeat, 1, hint_engines=(
                            mybir.EngineType.PE,
                            mybir.EngineType.DVE,
                            mybir.EngineType.Activation,
                            mybir.EngineType.SP,
                            mybir.EngineType.Pool)):
                        emit_body()
                else:
                    emit_body()

    nc.compile()
    return nc


def get_nc(repeat=1, cfg=None):
    key = f"nc{repeat}-{sorted((cfg or {}).items())}"
    if key not in _cached:
        _cached[key] = _build(repeat, cfg)
    return _cached[key]


def run(in_maps, trace=False, repeat=1, cfg=None, **kw):
    from concourse.bass_utils import run_bass_kernel_spmd

    nc = get_nc(repeat, cfg)
    return run_bass_kernel_spmd(nc, in_maps, list(range(B)), trace=trace, **kw)


def kernel(query, key, value, Wq, bq, Wk, bk, Wv, bv, Wo, bo):
    shared = {
        "Wq": np.ascontiguousarray(Wq, np.float32),
        "Wk": np.ascontiguousarray(Wk, np.float32),
        "Wv": np.ascontiguousarray(Wv, np.float32),
        "Wo": np.ascontiguousarray(Wo, np.float32),
        "bq": np.ascontiguousarray(bq, np.float32),
        "bk": np.ascontiguousarray(bk, np.float32),
        "bv": np.ascontiguousarray(bv, np.float32),
        "bo": np.ascontiguousarray(bo, np.float32),
    }
    in_maps = []
    for i in range(B):
        m = dict(shared)
        m["query"] = np.ascontiguousarray(query[i], np.float32)
        m["key"] = np.ascontiguousarray(key[i], np.float32)
        m["value"] = np.ascontiguousarray(value[i], np.float32)
        in_maps.append(m)
    res = run(in_maps)
    return np.stack([res.results[i]["out"] for i in range(B)], axis=0)


# revision 20
# speedup vs baseline: 1.8160x; 1.0194x over previous
"""v3: fine-grained weave of projections into attention heads.

Differences vs v1:
  - Phase A (transpose+projections) is emitted per token-group g, immediately
    followed by the full attention/normalize/out-proj for q supertile qt=g
    (which only depends on groups <= g). PE-heavy A work fills the PE idle
    of the ACT-bound attention phase.
  - One shared [128,512] PSUM pool (bufs=6) for transposes/projections/
    scores/bcast/out-proj + a dedicated [65,512] accumulation pool (bufs=2).
  - All biases are folded into DVE ops instead of K=1 matmuls:
      QT/KT: tensor_scalar_add with per-partition bias columns (PE-transposed
      from the bias rows once at startup)
      V/O: tensor_tensor add with pre-broadcast bias tiles (built once via a
      single ones-matmul each)
"""

import numpy as np

B = 8
L = 2048
D = 512
H = 8
DH = 64
NT = L // 128
NCH = D // 128
NQ = L // 512

_cached = {}


def _build(repeat=1, cfg=None):
    cfg = dict(cfg or {})
    PS512 = cfg.get("ps512", 2)
    SPS2 = cfg.get("sps2", 2)
    OPS = cfg.get("ops", 2)
    PEXP = cfg.get("pexp", 6)
    OSB = cfg.get("osb", 3)
    XIN = cfg.get("xin", 8)
    XT = cfg.get("xt", 10)
    import concourse.tile as tile
    from concourse import mybir, bacc
    from concourse.masks import make_identity

    f32 = mybir.dt.float32
    bf16 = mybir.dt.bfloat16
    f32r = mybir.dt.float32r

    nc = bacc.Bacc("TRN2", target_bir_lowering=False, debug=False)

    xq = nc.dram_tensor("query", [L, D], f32, kind="ExternalInput").ap()
    xk = nc.dram_tensor("key", [L, D], f32, kind="ExternalInput").ap()
    xv = nc.dram_tensor("value", [L, D], f32, kind="ExternalInput").ap()
    Wq = nc.dram_tensor("Wq", [D, D], f32, kind="ExternalInput").ap()
    Wk = nc.dram_tensor("Wk", [D, D], f32, kind="ExternalInput").ap()
    Wv = nc.dram_tensor("Wv", [D, D], f32, kind="ExternalInput").ap()
    Wo = nc.dram_tensor("Wo", [D, D], f32, kind="ExternalInput").ap()
    bq = nc.dram_tensor("bq", [D], f32, kind="ExternalInput").ap()
    bk = nc.dram_tensor("bk", [D], f32, kind="ExternalInput").ap()
    bv = nc.dram_tensor("bv", [D], f32, kind="ExternalInput").ap()
    bo = nc.dram_tensor("bo", [D], f32, kind="ExternalInput").ap()
    out = nc.dram_tensor("out", [L, D], f32, kind="ExternalOutput").ap()

    def r(ap):
        return ap.bitcast(f32r)

    with tile.TileContext(nc) as tc:
        with (
            tc.tile_pool(name="persist", bufs=1) as persist,
            tc.tile_pool(name="consts", bufs=1) as consts,
            tc.tile_pool(name="ps512", bufs=PS512, space="PSUM") as ps512,
            tc.tile_pool(name="sps2", bufs=SPS2, space="PSUM") as sps2_pool,
            tc.tile_pool(name="ops", bufs=OPS, space="PSUM") as ops_pool,
        ):
            # ---- constants ----
            ident = consts.tile([128, 128], f32, tag="ident")
            make_identity(nc, ident[:])
            tri = consts.tile([128, 128], bf16, tag="tri")
            nc.gpsimd.memset(tri[:], 0.0)
            nc.gpsimd.affine_select(
                out=tri[:], in_=tri[:], compare_op=mybir.AluOpType.is_gt,
                fill=1.0, base=0, pattern=[[-1, 128]], channel_multiplier=1,
            )
            tri_wide = consts.tile([128, 256], bf16, tag="tri_wide")
            nc.gpsimd.memset(tri_wide[:], 0.0)
            nc.gpsimd.affine_select(
                out=tri_wide[:], in_=tri_wide[:], compare_op=mybir.AluOpType.is_gt,
                fill=1.0, base=128, pattern=[[-1, 256]], channel_multiplier=1,
            )
            ones = consts.tile([1, 512], f32, tag="ones")
            nc.vector.memset(ones[:], 1.0)
            ones_t = consts.tile([128, 64], bf16, tag="ones_t")
            nc.vector.memset(ones_t[:], 1.0)

            # ---- weights / biases ----
            w_sb = {}
            b_row = {}
            with tc.tile_pool(name="wtmp", bufs=3) as wtmp_pool:
                for name, wdram in (("q", Wq), ("k", Wk), ("v", Wv), ("o", Wo)):
                    t = persist.tile([128, NCH, 512], f32, tag=f"W{name}",
                                     name=f"W{name}")
                    for c in range(NCH):
                        wt = wtmp_pool.tile([128, 512], f32, tag="wtmp",
                                            name="wtmp")
                        nc.gpsimd.dma_start(
                            wt[:], wdram[128 * c:128 * (c + 1), :])
                        nc.vector.tensor_copy(r(t[:, c, :]), wt[:])
                    w_sb[name] = t
                for name, bdram in (("q", bq), ("k", bk), ("v", bv), ("o", bo)):
                    t = wtmp_pool.tile([1, 512], f32, tag=f"b{name}",
                                       name=f"b{name}", bufs=1)
                    nc.gpsimd.dma_start(t[:], bdram[None, :])
                    b_row[name] = t
                # per-partition bias columns for q/k (dout on partitions)
                bcol = {}
                for name in ("q", "k"):
                    bc_t = consts.tile([128, NCH], f32, tag=f"bcol{name}",
                                       name=f"bcol{name}")
                    for c in range(NCH):
                        tp = ps512.tile([128, 512], f32, tag="ps512", name="ps512")
                        nc.tensor.transpose(
                            tp[:, 0:1], b_row[name][0:1, 128 * c:128 * (c + 1)],
                            ident[0:1, 0:1])
                        nc.vector.tensor_copy(bc_t[:, c:c + 1], tp[:, 0:1])
                    bcol[name] = bc_t
                # broadcast bias tiles for v (head-interleaved) and o (natural)
                bvb = consts.tile([128, H, DH], f32, tag="bvb", name="bvb")
                bob = consts.tile([128, 512], f32, tag="bob", name="bob")
                for dst, row in ((bvb, b_row["v"]), (bob, b_row["o"])):
                    rowr = wtmp_pool.tile([1, 512], f32, tag="browr",
                                          name="browr", bufs=2)
                    nc.vector.tensor_copy(r(rowr[:]), row[:])
                    tp = ps512.tile([128, 512], f32, tag="ps512", name="ps512")
                    nc.tensor.matmul(tp[:], r(ones[0:1, 0:128]), r(rowr[:]),
                                     start=True, stop=True)
                    if dst is bvb:
                        nc.vector.tensor_copy(
                            dst[:], tp[:].rearrange("p (h d) -> p h d", h=H))
                    else:
                        nc.vector.tensor_copy(dst[:], tp[:])

            # ---- persistent activations ----
            kt_sb = [persist.tile([128, L], f32, tag=f"KT{c}", name=f"KT{c}")
                     for c in range(NCH)]
            v_sb = [persist.tile([128, H, DH + 1], bf16, tag=f"V{t}",
                        name=f"V{t}") for t in range(NT)]
            stage = [persist.tile([128, L], f32, tag=f"stage{c}", name=f"stage{c}")
                     for c in range(NCH)]

            with (
                tc.tile_pool(name="xin", bufs=XIN) as xin_pool,
                tc.tile_pool(name="qtg", bufs=2) as qtg_pool,
                tc.tile_pool(name="xt", bufs=XT) as xt_pool,
                tc.tile_pool(name="pexp", bufs=PEXP) as p_pool,
                tc.tile_pool(name="norm", bufs=1) as norm_pool,
                tc.tile_pool(name="osb", bufs=OSB) as o_pool,
            ):
                def emit_a_pieces(g):
                    """Return (qt_g, [thunk, ...]) -- pieces of the
                    transpose+projection work for token group g, to be woven
                    between attention heads of the previous supertile."""
                    qt_g = [qtg_pool.tile([128, 512], f32, tag=f"qtg{c}",
                                          name=f"qtg{c}") for c in range(NCH)]
                    pieces = []
                    for tname_, xdram_ in (("k", xk), ("v", xv), ("q", xq)):
                        pieces.append(
                            lambda tname=tname_, xdram=xdram_: emit_a_tensor(
                                g, tname, xdram, qt_g))
                    return qt_g, pieces

                def emit_a_tensor(g, tname, xdram, qt_g):
                    if True:
                        xtiles = []
                        for j in range(4):
                            t0 = 4 * g + j
                            xt_in = xin_pool.tile([128, 512], f32, tag="xin",
                                                  name="xin")
                            nc.sync.dma_start(
                                xt_in[:], xdram[128 * t0:128 * (t0 + 1), :])
                            xtiles.append(xt_in)
                        xt_c = []
                        for c in range(NCH):
                            ps = ps512.tile([128, 512], f32, tag="ps512",
                                            name="ps512")
                            for j in range(4):
                                nc.tensor.transpose(
                                    ps[:, 128 * j:128 * (j + 1)],
                                    xtiles[j][:, 128 * c:128 * (c + 1)],
                                    ident[:],
                                )
                            sb = xt_pool.tile([128, 512], f32, tag="xt", name="xt")
                            nc.vector.tensor_copy(r(sb[:]), ps[:])
                            xt_c.append(sb)
                        if tname in ("q", "k"):
                            for co in range(NCH):
                                pp = ps512.tile([128, 512], f32, tag="ps512",
                                                name="ps512")
                                for ci in range(NCH):
                                    nc.tensor.matmul(
                                        pp[:],
                                        r(w_sb[tname][
                                            :, ci, 128 * co:128 * (co + 1)]),
                                        r(xt_c[ci][:]),
                                        start=(ci == 0), stop=(ci == NCH - 1),
                                    )
                                if tname == "q":
                                    nc.vector.tensor_scalar_add(
                                        r(qt_g[co][:]), pp[:],
                                        bcol["q"][:, co:co + 1])
                                else:
                                    nc.vector.tensor_scalar_add(
                                        r(kt_sb[co][:, 512 * g:512 * (g + 1)]),
                                        pp[:], bcol["k"][:, co:co + 1])
                        else:
                            for j in range(4):
                                t0 = 4 * g + j
                                pv = ps512.tile([128, 512], f32, tag="ps512",
                                                name="ps512")
                                for ci in range(NCH):
                                    nc.tensor.matmul(
                                        pv[:],
                                        r(xt_c[ci][:, 128 * j:128 * (j + 1)]),
                                        r(w_sb["v"][:, ci, :]),
                                        start=(ci == 0), stop=(ci == NCH - 1),
                                    )
                                nc.vector.tensor_add(
                                    v_sb[t0][:, :, 0:DH],
                                    pv[:].rearrange("p (h d) -> p h d", h=H),
                                    bvb[:],
                                )
                                nc.gpsimd.memset(v_sb[t0][:, :, DH:DH + 1], 1.0)

                def emit_b_qt(qt, qt_g, weave=()):
                    weave = list(weave)
                    """Attention + normalize + out-proj for q supertile qt."""
                    kmax = 4 * qt + 4
                    stg = norm_pool.tile([128, 1536], f32, tag="stg", name="stg")
                    for h in range(H):
                        ch, prow = h // 2, 64 * (h % 2)
                        kth = kt_sb[ch]
                        qth = qt_g[ch]
                        po = ops_pool.tile([65, 512], f32, tag="ops", name="ops")
                        for pi in range(kmax // 2):
                            cc = (2 * pi, 2 * pi + 1)
                            mm = [c - 4 * qt for c in cc]
                            # scores (f32r) need matmul N >= 256; the bf16
                            # attn@V matmul has no such constraint
                            js0 = [0 if m < 1 else (128 * m if m < 3 else 256)
                                   for m in mm]
                            jv0 = [0 if m < 1 else 128 * m for m in mm]
                            ps = sps2_pool.tile([128, 1024], f32, tag="sps2",
                                                name="sps2")
                            pt = p_pool.tile([128, 1024], bf16, tag="pexp",
                                             name="pexp")
                            for k in range(2):
                                nc.tensor.matmul(
                                    ps[:, 512 * k + js0[k]:512 * (k + 1)],
                                    r(kth[prow:prow + DH,
                                          128 * cc[k]:128 * (cc[k] + 1)]),
                                    r(qth[prow:prow + DH, js0[k]:512]),
                                    start=True, stop=True,
                                )
                            if mm[1] < 0:  # both chunks full-width: one exp
                                nc.scalar.activation(
                                    pt[:], ps[:],
                                    mybir.ActivationFunctionType.Exp,
                                    scale=0.125,
                                )
                            else:
                                for k in range(2):
                                    nc.scalar.activation(
                                        pt[:, 512 * k + jv0[k]:512 * (k + 1)],
                                        ps[:, 512 * k + jv0[k]:512 * (k + 1)],
                                        mybir.ActivationFunctionType.Exp,
                                        scale=0.125,
                                    )
                            for k in range(2):
                                m = mm[k]
                                if m >= 0:
                                    nc.vector.tensor_mul(
                                        pt[:, 512 * k + 128 * m:
                                           512 * k + 128 * (m + 1)],
                                        pt[:, 512 * k + 128 * m:
                                           512 * k + 128 * (m + 1)], tri[:])
                                nc.tensor.matmul(
                                    po[:, jv0[k]:512],
                                    v_sb[cc[k]][:, h, :],
                                    pt[:, 512 * k + jv0[k]:512 * (k + 1)],
                                    start=(cc[k] == 0),
                                    stop=(cc[k] == kmax - 1),
                                )
                        nc.vector.tensor_copy(
                            r(stage[ch][prow:prow + DH,
                                        512 * qt:512 * (qt + 1)]),
                            po[0:DH, :])
                        nc.vector.tensor_copy(
                            stg[32 * (h % 3):32 * (h % 3) + 1,
                                512 * (h // 3):512 * (h // 3) + 512],
                            po[DH:DH + 1, :])
                        if weave and h in (2, 4, 6):
                            weave.pop(0)()
                    rstg = norm_pool.tile([128, 1536], f32, tag="rstg",
                                          name="rstg")
                    nc.vector.reciprocal_approx_fast(out=rstg[:], in_=stg[:])
                    rbf = norm_pool.tile([128, 1536], bf16, tag="rbf", name="rbf")
                    nc.vector.tensor_copy(rbf[:], rstg[:])
                    for ch in range(NCH):
                        bcp = ps512.tile([128, 512], f32, tag="ps512",
                                         name="ps512")
                        for sub in range(2):
                            hh = 2 * ch + sub
                            pp0 = 32 * (hh % 3)
                            fo = 512 * (hh // 3)
                            nc.tensor.matmul(
                                bcp[64 * sub:64 * sub + 64, :],
                                ones_t[pp0:pp0 + 1, 0:64],
                                rbf[pp0:pp0 + 1, fo:fo + 512],
                                start=True, stop=True,
                            )
                        nc.vector.tensor_mul(
                            r(stage[ch][:, 512 * qt:512 * (qt + 1)]),
                            stage[ch][:, 512 * qt:512 * (qt + 1)],
                            bcp[:],
                        )
                    for i in range(4 * qt, 4 * qt + 4):
                        pout = ps512.tile([128, 512], f32, tag="ps512",
                                          name="ps512")
                        for ch in range(NCH):
                            nc.tensor.matmul(
                                pout[:],
                                r(stage[ch][:, 128 * i:128 * (i + 1)]),
                                r(w_sb["o"][:, ch, :]),
                                start=(ch == 0), stop=(ch == NCH - 1),
                            )
                        ot = o_pool.tile([128, 512], f32, tag="osb", name="osb")
                        nc.vector.tensor_add(ot[:], pout[:], bob[:])
                        nc.sync.dma_start(out[128 * i:128 * (i + 1), :], ot[:])
                    for w in weave:
                        w()

                def emit_body():
                    qt_g, pieces = emit_a_pieces(0)
                    for p in pieces:
                        p()
                    for g in range(NQ):
                        if g + 1 < NQ:
                            qt_next, weave = emit_a_pieces(g + 1)
                        else:
                            qt_next, weave = None, ()
                        emit_b_qt(g, qt_g, weave)
                        qt_g = qt_next

                if repeat > 1:
                    with tc.For_i(0, repeat, 1, hint_engines=(
                            mybir.EngineType.PE,
                            mybir.EngineType.DVE,
                            mybir.EngineType.Activation,
                            mybir.EngineType.SP,
                            mybir.EngineType.Pool)):
                        emit_body()
                else:
                    emit_body()

    nc.compile()
    return nc


def get_nc(repeat=1, cfg=None):
    key = f"nc{repeat}-{sorted((cfg or {}).items())}"
    if key not in _cached:
        _cached[key] = _build(repeat, cfg)
    return _cached[key]


def run(in_maps, trace=False, repeat=1, cfg=None, **kw):
    from concourse.bass_utils import run_bass_kernel_spmd

    nc = get_nc(repeat, cfg)
    return run_bass_kernel_spmd(nc, in_maps, list(range(B)), trace=trace, **kw)


def kernel(query, key, value, Wq, bq, Wk, bk, Wv, bv, Wo, bo):
    shared = {
        "Wq": np.ascontiguousarray(Wq, np.float32),
        "Wk": np.ascontiguousarray(Wk, np.float32),
        "Wv": np.ascontiguousarray(Wv, np.float32),
        "Wo": np.ascontiguousarray(Wo, np.float32),
        "bq": np.ascontiguousarray(bq, np.float32),
        "bk": np.ascontiguousarray(bk, np.float32),
        "bv": np.ascontiguousarray(bv, np.float32),
        "bo": np.ascontiguousarray(bo, np.float32),
    }
    in_maps = []
    for i in range(B):
        m = dict(shared)
        m["query"] = np.ascontiguousarray(query[i], np.float32)
        m["key"] = np.ascontiguousarray(key[i], np.float32)
        m["value"] = np.ascontiguousarray(value[i], np.float32)
        in_maps.append(m)
    res = run(in_maps)
    return np.stack([res.results[i]["out"] for i in range(B)], axis=0)


# revision 21
# speedup vs baseline: 2.1299x; 1.1728x over previous
"""v3: fine-grained weave of projections into attention heads.

Differences vs v1:
  - Phase A (transpose+projections) is emitted per token-group g, immediately
    followed by the full attention/normalize/out-proj for q supertile qt=g
    (which only depends on groups <= g). PE-heavy A work fills the PE idle
    of the ACT-bound attention phase.
  - One shared [128,512] PSUM pool (bufs=6) for transposes/projections/
    scores/bcast/out-proj + a dedicated [65,512] accumulation pool (bufs=2).
  - All biases are folded into DVE ops instead of K=1 matmuls:
      QT/KT: tensor_scalar_add with per-partition bias columns (PE-transposed
      from the bias rows once at startup)
      V/O: tensor_tensor add with pre-broadcast bias tiles (built once via a
      single ones-matmul each)
"""

import numpy as np

B = 8
L = 2048
D = 512
H = 8
DH = 64
NT = L // 128
NCH = D // 128
NQ = L // 512

_cached = {}


def _build(repeat=1, cfg=None):
    cfg = dict(cfg or {})
    PS512 = cfg.get("ps512", 2)
    SPS2 = cfg.get("sps2", 2)
    OPS = cfg.get("ops", 2)
    PEXP = cfg.get("pexp", 6)
    OSB = cfg.get("osb", 3)
    XIN = cfg.get("xin", 8)
    XT = cfg.get("xt", 10)
    import concourse.tile as tile
    from concourse import mybir, bacc
    from concourse.masks import make_identity

    f32 = mybir.dt.float32
    bf16 = mybir.dt.bfloat16
    f32r = mybir.dt.float32r

    nc = bacc.Bacc("TRN2", target_bir_lowering=False, debug=False)

    xq = nc.dram_tensor("query", [L, D], f32, kind="ExternalInput").ap()
    xk = nc.dram_tensor("key", [L, D], f32, kind="ExternalInput").ap()
    xv = nc.dram_tensor("value", [L, D], f32, kind="ExternalInput").ap()
    Wq = nc.dram_tensor("Wq", [D, D], f32, kind="ExternalInput").ap()
    Wk = nc.dram_tensor("Wk", [D, D], f32, kind="ExternalInput").ap()
    Wv = nc.dram_tensor("Wv", [D, D], f32, kind="ExternalInput").ap()
    Wo = nc.dram_tensor("Wo", [D, D], f32, kind="ExternalInput").ap()
    bq = nc.dram_tensor("bq", [D], f32, kind="ExternalInput").ap()
    bk = nc.dram_tensor("bk", [D], f32, kind="ExternalInput").ap()
    bv = nc.dram_tensor("bv", [D], f32, kind="ExternalInput").ap()
    bo = nc.dram_tensor("bo", [D], f32, kind="ExternalInput").ap()
    out = nc.dram_tensor("out", [L, D], f32, kind="ExternalOutput").ap()

    def r(ap):
        return ap.bitcast(f32r)

    with tile.TileContext(nc) as tc:
        with (
            tc.tile_pool(name="persist", bufs=1) as persist,
            tc.tile_pool(name="consts", bufs=1) as consts,
            tc.tile_pool(name="ps512", bufs=PS512, space="PSUM") as ps512,
            tc.tile_pool(name="sps2", bufs=SPS2, space="PSUM") as sps2_pool,
            tc.tile_pool(name="ops", bufs=OPS, space="PSUM") as ops_pool,
        ):
            # ---- constants ----
            ident = consts.tile([128, 128], f32, tag="ident")
            make_identity(nc, ident[:])
            tri = consts.tile([128, 128], bf16, tag="tri")
            nc.gpsimd.memset(tri[:], 0.0)
            nc.gpsimd.affine_select(
                out=tri[:], in_=tri[:], compare_op=mybir.AluOpType.is_gt,
                fill=1.0, base=0, pattern=[[-1, 128]], channel_multiplier=1,
            )
            tri_wide = consts.tile([128, 256], bf16, tag="tri_wide")
            nc.gpsimd.memset(tri_wide[:], 0.0)
            nc.gpsimd.affine_select(
                out=tri_wide[:], in_=tri_wide[:], compare_op=mybir.AluOpType.is_gt,
                fill=1.0, base=128, pattern=[[-1, 256]], channel_multiplier=1,
            )
            ones = consts.tile([1, 512], f32, tag="ones")
            nc.vector.memset(ones[:], 1.0)
            ones_t = consts.tile([128, 64], bf16, tag="ones_t")
            nc.vector.memset(ones_t[:], 1.0)

            # ---- weights / biases ----
            w_sb = {}
            b_row = {}
            with tc.tile_pool(name="wtmp", bufs=3) as wtmp_pool:
                for name, wdram in (("q", Wq), ("k", Wk), ("v", Wv), ("o", Wo)):
                    t = persist.tile([128, NCH, 512], f32, tag=f"W{name}",
                                     name=f"W{name}")
                    for c in range(NCH):
                        wt = wtmp_pool.tile([128, 512], f32, tag="wtmp",
                                            name="wtmp")
                        nc.gpsimd.dma_start(
                            wt[:], wdram[128 * c:128 * (c + 1), :])
                        nc.vector.tensor_copy(r(t[:, c, :]), wt[:])
                    w_sb[name] = t
                for name, bdram in (("q", bq), ("k", bk), ("v", bv), ("o", bo)):
                    t = wtmp_pool.tile([1, 512], f32, tag=f"b{name}",
                                       name=f"b{name}", bufs=1)
                    nc.gpsimd.dma_start(t[:], bdram[None, :])
                    b_row[name] = t
                # per-partition bias columns for q/k (dout on partitions)
                bcol = {}
                for name in ("q", "k"):
                    bc_t = consts.tile([128, NCH], f32, tag=f"bcol{name}",
                                       name=f"bcol{name}")
                    for c in range(NCH):
                        tp = ps512.tile([128, 512], f32, tag="ps512", name="ps512")
                        nc.tensor.transpose(
                            tp[:, 0:1], b_row[name][0:1, 128 * c:128 * (c + 1)],
                            ident[0:1, 0:1])
                        nc.vector.tensor_copy(bc_t[:, c:c + 1], tp[:, 0:1])
                    bcol[name] = bc_t
                # broadcast bias tiles for v (head-interleaved) and o (natural)
                bvb = consts.tile([128, H, DH], f32, tag="bvb", name="bvb")
                bob = consts.tile([128, 512], f32, tag="bob", name="bob")
                for dst, row in ((bvb, b_row["v"]), (bob, b_row["o"])):
                    rowr = wtmp_pool.tile([1, 512], f32, tag="browr",
                                          name="browr", bufs=2)
                    nc.vector.tensor_copy(r(rowr[:]), row[:])
                    tp = ps512.tile([128, 512], f32, tag="ps512", name="ps512")
                    nc.tensor.matmul(tp[:], r(ones[0:1, 0:128]), r(rowr[:]),
                                     start=True, stop=True)
                    if dst is bvb:
                        nc.vector.tensor_copy(
                            dst[:], tp[:].rearrange("p (h d) -> p h d", h=H))
                    else:
                        nc.vector.tensor_copy(dst[:], tp[:])

            # ---- persistent activations ----
            kt_sb = [persist.tile([128, L], f32, tag=f"KT{c}", name=f"KT{c}")
                     for c in range(NCH)]
            v_sb = [persist.tile([128, H, DH + 1], bf16, tag=f"V{t}",
                        name=f"V{t}") for t in range(NT)]
            stage = [persist.tile([128, L], f32, tag=f"stage{c}", name=f"stage{c}")
                     for c in range(NCH)]

            with (
                tc.tile_pool(name="xin", bufs=XIN) as xin_pool,
                tc.tile_pool(name="qtg", bufs=2) as qtg_pool,
                tc.tile_pool(name="xt", bufs=XT) as xt_pool,
                tc.tile_pool(name="pexp", bufs=PEXP) as p_pool,
                tc.tile_pool(name="norm", bufs=1) as norm_pool,
                tc.tile_pool(name="osb", bufs=OSB) as o_pool,
            ):
                def emit_a_pieces(g):
                    """Return (qt_g, [thunk, ...]) -- pieces of the
                    transpose+projection work for token group g, to be woven
                    between attention heads of the previous supertile."""
                    qt_g = [qtg_pool.tile([128, 512], f32, tag=f"qtg{c}",
                                          name=f"qtg{c}") for c in range(NCH)]
                    pieces = []
                    for tname_, xdram_ in (("k", xk), ("v", xv), ("q", xq)):
                        pieces.append(
                            lambda tname=tname_, xdram=xdram_: emit_a_tensor(
                                g, tname, xdram, qt_g))
                    return qt_g, pieces

                def emit_a_tensor(g, tname, xdram, qt_g):
                    if True:
                        xtiles = []
                        for j in range(4):
                            t0 = 4 * g + j
                            xt_in = xin_pool.tile([128, 512], f32, tag="xin",
                                                  name="xin")
                            nc.sync.dma_start(
                                xt_in[:], xdram[128 * t0:128 * (t0 + 1), :])
                            xtiles.append(xt_in)
                        xt_c = []
                        for c in range(NCH):
                            ps = ps512.tile([128, 512], f32, tag="ps512",
                                            name="ps512")
                            for j in range(4):
                                nc.tensor.transpose(
                                    ps[:, 128 * j:128 * (j + 1)],
                                    xtiles[j][:, 128 * c:128 * (c + 1)],
                                    ident[:],
                                )
                            sb = xt_pool.tile([128, 512], f32, tag="xt", name="xt")
                            nc.vector.tensor_copy(r(sb[:]), ps[:])
                            xt_c.append(sb)
                        if tname in ("q", "k"):
                            for co in range(NCH):
                                pp = ps512.tile([128, 512], f32, tag="ps512",
                                                name="ps512")
                                for ci in range(NCH):
                                    nc.tensor.matmul(
                                        pp[:],
                                        r(w_sb[tname][
                                            :, ci, 128 * co:128 * (co + 1)]),
                                        r(xt_c[ci][:]),
                                        start=(ci == 0), stop=(ci == NCH - 1),
                                    )
                                if tname == "q":
                                    nc.vector.tensor_scalar_add(
                                        r(qt_g[co][:]), pp[:],
                                        bcol["q"][:, co:co + 1])
                                else:
                                    nc.vector.tensor_scalar_add(
                                        r(kt_sb[co][:, 512 * g:512 * (g + 1)]),
                                        pp[:], bcol["k"][:, co:co + 1])
                        else:
                            for j in range(4):
                                t0 = 4 * g + j
                                pv = ps512.tile([128, 512], f32, tag="ps512",
                                                name="ps512")
                                for ci in range(NCH):
                                    nc.tensor.matmul(
                                        pv[:],
                                        r(xt_c[ci][:, 128 * j:128 * (j + 1)]),
                                        r(w_sb["v"][:, ci, :]),
                                        start=(ci == 0), stop=(ci == NCH - 1),
                                    )
                                nc.vector.tensor_add(
                                    v_sb[t0][:, :, 0:DH],
                                    pv[:].rearrange("p (h d) -> p h d", h=H),
                                    bvb[:],
                                )
                                nc.gpsimd.memset(v_sb[t0][:, :, DH:DH + 1], 1.0)

                def emit_b_qt(qt, qt_g, weave=()):
                    weave = list(weave)
                    kmax = 4 * qt + 4
                    stg = norm_pool.tile([128, 1536], f32, tag="stg", name="stg")
                    for hp in range(H // 2):
                        # head pair (2hp, 2hp+1): score matmuls alternate
                        # PE row-halves (prow 0/64) -> array-level overlap
                        ch = hp
                        kth = kt_sb[ch]
                        qth = qt_g[ch]
                        po = [ops_pool.tile([65, 512], f32, tag="ops",
                                            name="ops") for _ in range(2)]
                        for c in range(kmax):
                            m = c - 4 * qt
                            js0 = 0 if m < 1 else (128 * m if m < 3 else 256)
                            jv0 = 0 if m < 1 else 128 * m
                            ps = sps2_pool.tile([128, 1024], f32, tag="sps2",
                                                name="sps2")
                            pt = p_pool.tile([128, 1024], bf16, tag="pexp",
                                             name="pexp")
                            for k in range(2):
                                prow = 64 * k
                                nc.tensor.matmul(
                                    ps[:, 512 * k + js0:512 * (k + 1)],
                                    r(kth[prow:prow + DH,
                                          128 * c:128 * (c + 1)]),
                                    r(qth[prow:prow + DH, js0:512]),
                                    start=True, stop=True,
                                )
                            if m < 0:
                                nc.scalar.activation(
                                    pt[:], ps[:],
                                    mybir.ActivationFunctionType.Exp,
                                    scale=0.125,
                                )
                            else:
                                for k in range(2):
                                    nc.scalar.activation(
                                        pt[:, 512 * k + jv0:512 * (k + 1)],
                                        ps[:, 512 * k + jv0:512 * (k + 1)],
                                        mybir.ActivationFunctionType.Exp,
                                        scale=0.125,
                                    )
                            if m >= 0:
                                for k in range(2):
                                    nc.vector.tensor_mul(
                                        pt[:, 512 * k + 128 * m:
                                           512 * k + 128 * (m + 1)],
                                        pt[:, 512 * k + 128 * m:
                                           512 * k + 128 * (m + 1)], tri[:])
                            for k in range(2):
                                nc.tensor.matmul(
                                    po[k][:, jv0:512],
                                    v_sb[c][:, 2 * hp + k, :],
                                    pt[:, 512 * k + jv0:512 * (k + 1)],
                                    start=(c == 0), stop=(c == kmax - 1),
                                )
                        for k in range(2):
                            h = 2 * hp + k
                            prow = 64 * k
                            nc.vector.tensor_copy(
                                r(stage[ch][prow:prow + DH,
                                            512 * qt:512 * (qt + 1)]),
                                po[k][0:DH, :])
                            nc.vector.tensor_copy(
                                stg[32 * (h % 3):32 * (h % 3) + 1,
                                    512 * (h // 3):512 * (h // 3) + 512],
                                po[k][DH:DH + 1, :])
                        if weave and hp in (1, 2, 3):
                            weave.pop(0)()
                    rstg = norm_pool.tile([128, 1536], f32, tag="rstg",
                                          name="rstg")
                    nc.vector.reciprocal_approx_fast(out=rstg[:], in_=stg[:])
                    rbf = norm_pool.tile([128, 1536], bf16, tag="rbf", name="rbf")
                    nc.vector.tensor_copy(rbf[:], rstg[:])
                    for ch in range(NCH):
                        bcp = ps512.tile([128, 512], f32, tag="ps512",
                                         name="ps512")
                        for sub in range(2):
                            hh = 2 * ch + sub
                            pp0 = 32 * (hh % 3)
                            fo = 512 * (hh // 3)
                            nc.tensor.matmul(
                                bcp[64 * sub:64 * sub + 64, :],
                                ones_t[pp0:pp0 + 1, 0:64],
                                rbf[pp0:pp0 + 1, fo:fo + 512],
                                start=True, stop=True,
                            )
                        nc.vector.tensor_mul(
                            r(stage[ch][:, 512 * qt:512 * (qt + 1)]),
                            stage[ch][:, 512 * qt:512 * (qt + 1)],
                            bcp[:],
                        )
                    for i in range(4 * qt, 4 * qt + 4):
                        pout = ps512.tile([128, 512], f32, tag="ps512",
                                          name="ps512")
                        for ch in range(NCH):
                            nc.tensor.matmul(
                                pout[:],
                                r(stage[ch][:, 128 * i:128 * (i + 1)]),
                                r(w_sb["o"][:, ch, :]),
                                start=(ch == 0), stop=(ch == NCH - 1),
                            )
                        ot = o_pool.tile([128, 512], f32, tag="osb", name="osb")
                        nc.vector.tensor_add(ot[:], pout[:], bob[:])
                        nc.sync.dma_start(out[128 * i:128 * (i + 1), :], ot[:])
                    for w in weave:
                        w()

                def emit_body():
                    qt_g, pieces = emit_a_pieces(0)
                    for p in pieces:
                        p()
                    for g in range(NQ):
                        if g + 1 < NQ:
                            qt_next, weave = emit_a_pieces(g + 1)
                        else:
                            qt_next, weave = None, ()
                        emit_b_qt(g, qt_g, weave)
                        qt_g = qt_next

                if repeat > 1:
                    with tc.For_i(0, repeat, 1, hint_engines=(
                            mybir.EngineType.PE,
                            mybir.EngineType.DVE,
                            mybir.EngineType.Activation,
                            mybir.EngineType.SP,
                            mybir.EngineType.Pool)):
                        emit_body()
                else:
                    emit_body()

    nc.compile()
    return nc


def get_nc(repeat=1, cfg=None):
    key = f"nc{repeat}-{sorted((cfg or {}).items())}"
    if key not in _cached:
        _cached[key] = _build(repeat, cfg)
    return _cached[key]


def run(in_maps, trace=False, repeat=1, cfg=None, **kw):
    from concourse.bass_utils import run_bass_kernel_spmd

    nc = get_nc(repeat, cfg)
    return run_bass_kernel_spmd(nc, in_maps, list(range(B)), trace=trace, **kw)


def kernel(query, key, value, Wq, bq, Wk, bk, Wv, bv, Wo, bo):
    shared = {
        "Wq": np.ascontiguousarray(Wq, np.float32),
        "Wk": np.ascontiguousarray(Wk, np.float32),
        "Wv": np.ascontiguousarray(Wv, np.float32),
        "Wo": np.ascontiguousarray(Wo, np.float32),
        "bq": np.ascontiguousarray(bq, np.float32),
        "bk": np.ascontiguousarray(bk, np.float32),
        "bv": np.ascontiguousarray(bv, np.float32),
        "bo": np.ascontiguousarray(bo, np.float32),
    }
    in_maps = []
    for i in range(B):
        m = dict(shared)
        m["query"] = np.ascontiguousarray(query[i], np.float32)
        m["key"] = np.ascontiguousarray(key[i], np.float32)
        m["value"] = np.ascontiguousarray(value[i], np.float32)
        in_maps.append(m)
    res = run(in_maps)
    return np.stack([res.results[i]["out"] for i in range(B)], axis=0)


# revision 23
# speedup vs baseline: 2.1367x; 1.0032x over previous
"""v3: fine-grained weave of projections into attention heads.

Differences vs v1:
  - Phase A (transpose+projections) is emitted per token-group g, immediately
    followed by the full attention/normalize/out-proj for q supertile qt=g
    (which only depends on groups <= g). PE-heavy A work fills the PE idle
    of the ACT-bound attention phase.
  - One shared [128,512] PSUM pool (bufs=6) for transposes/projections/
    scores/bcast/out-proj + a dedicated [65,512] accumulation pool (bufs=2).
  - All biases are folded into DVE ops instead of K=1 matmuls:
      QT/KT: tensor_scalar_add with per-partition bias columns (PE-transposed
      from the bias rows once at startup)
      V/O: tensor_tensor add with pre-broadcast bias tiles (built once via a
      single ones-matmul each)
"""

import numpy as np

B = 8
L = 2048
D = 512
H = 8
DH = 64
NT = L // 128
NCH = D // 128
NQ = L // 512

_cached = {}


def _build(repeat=1, cfg=None):
    cfg = dict(cfg or {})
    PS512 = cfg.get("ps512", 2)
    SPS2 = cfg.get("sps2", 2)
    OPS = cfg.get("ops", 2)
    PEXP = cfg.get("pexp", 6)
    OSB = cfg.get("osb", 3)
    XIN = cfg.get("xin", 8)
    XT = cfg.get("xt", 10)
    import concourse.tile as tile
    from concourse import mybir, bacc
    from concourse.masks import make_identity

    f32 = mybir.dt.float32
    bf16 = mybir.dt.bfloat16
    f32r = mybir.dt.float32r

    nc = bacc.Bacc("TRN2", target_bir_lowering=False, debug=False)

    xq = nc.dram_tensor("query", [L, D], f32, kind="ExternalInput").ap()
    xk = nc.dram_tensor("key", [L, D], f32, kind="ExternalInput").ap()
    xv = nc.dram_tensor("value", [L, D], f32, kind="ExternalInput").ap()
    Wq = nc.dram_tensor("Wq", [D, D], f32, kind="ExternalInput").ap()
    Wk = nc.dram_tensor("Wk", [D, D], f32, kind="ExternalInput").ap()
    Wv = nc.dram_tensor("Wv", [D, D], f32, kind="ExternalInput").ap()
    Wo = nc.dram_tensor("Wo", [D, D], f32, kind="ExternalInput").ap()
    bq = nc.dram_tensor("bq", [D], f32, kind="ExternalInput").ap()
    bk = nc.dram_tensor("bk", [D], f32, kind="ExternalInput").ap()
    bv = nc.dram_tensor("bv", [D], f32, kind="ExternalInput").ap()
    bo = nc.dram_tensor("bo", [D], f32, kind="ExternalInput").ap()
    out = nc.dram_tensor("out", [L, D], f32, kind="ExternalOutput").ap()

    def r(ap):
        return ap.bitcast(f32r)

    with tile.TileContext(nc) as tc:
        with (
            tc.tile_pool(name="persist", bufs=1) as persist,
            tc.tile_pool(name="consts", bufs=1) as consts,
            tc.tile_pool(name="ps512", bufs=PS512, space="PSUM") as ps512,
            tc.tile_pool(name="sps2", bufs=SPS2, space="PSUM") as sps2_pool,
            tc.tile_pool(name="ops", bufs=OPS, space="PSUM") as ops_pool,
        ):
            # ---- constants ----
            ident = consts.tile([128, 128], f32, tag="ident")
            make_identity(nc, ident[:])
            tri = consts.tile([128, 128], bf16, tag="tri")
            nc.gpsimd.memset(tri[:], 0.0)
            nc.gpsimd.affine_select(
                out=tri[:], in_=tri[:], compare_op=mybir.AluOpType.is_gt,
                fill=1.0, base=0, pattern=[[-1, 128]], channel_multiplier=1,
            )
            tri_wide = consts.tile([128, 256], bf16, tag="tri_wide")
            nc.gpsimd.memset(tri_wide[:], 0.0)
            nc.gpsimd.affine_select(
                out=tri_wide[:], in_=tri_wide[:], compare_op=mybir.AluOpType.is_gt,
                fill=1.0, base=128, pattern=[[-1, 256]], channel_multiplier=1,
            )
            ones = consts.tile([1, 512], f32, tag="ones")
            nc.vector.memset(ones[:], 1.0)
            ones_t = consts.tile([128, 64], bf16, tag="ones_t")
            nc.vector.memset(ones_t[:], 1.0)

            # ---- weights / biases ----
            w_sb = {}
            b_row = {}
            with tc.tile_pool(name="wtmp", bufs=3) as wtmp_pool:
                for name, wdram in (("q", Wq), ("k", Wk), ("v", Wv), ("o", Wo)):
                    t = persist.tile([128, NCH, 512], f32, tag=f"W{name}",
                                     name=f"W{name}")
                    for c in range(NCH):
                        wt = wtmp_pool.tile([128, 512], f32, tag="wtmp",
                                            name="wtmp")
                        nc.gpsimd.dma_start(
                            wt[:], wdram[128 * c:128 * (c + 1), :])
                        nc.vector.tensor_copy(r(t[:, c, :]), wt[:])
                    w_sb[name] = t
                for name, bdram in (("q", bq), ("k", bk), ("v", bv), ("o", bo)):
                    t = wtmp_pool.tile([1, 512], f32, tag=f"b{name}",
                                       name=f"b{name}", bufs=1)
                    nc.gpsimd.dma_start(t[:], bdram[None, :])
                    b_row[name] = t
                # per-partition bias columns for q/k (dout on partitions)
                bcol = {}
                for name in ("q", "k"):
                    bc_t = consts.tile([128, NCH], f32, tag=f"bcol{name}",
                                       name=f"bcol{name}")
                    for c in range(NCH):
                        tp = ps512.tile([128, 512], f32, tag="ps512", name="ps512")
                        nc.tensor.transpose(
                            tp[:, 0:1], b_row[name][0:1, 128 * c:128 * (c + 1)],
                            ident[0:1, 0:1])
                        nc.vector.tensor_copy(bc_t[:, c:c + 1], tp[:, 0:1])
                    bcol[name] = bc_t
                # broadcast bias tiles for v (head-interleaved) and o (natural)
                bvb = consts.tile([128, H, DH], f32, tag="bvb", name="bvb")
                bob = consts.tile([128, 512], f32, tag="bob", name="bob")
                for dst, row in ((bvb, b_row["v"]), (bob, b_row["o"])):
                    rowr = wtmp_pool.tile([1, 512], f32, tag="browr",
                                          name="browr", bufs=2)
                    nc.vector.tensor_copy(r(rowr[:]), row[:])
                    tp = ps512.tile([128, 512], f32, tag="ps512", name="ps512")
                    nc.tensor.matmul(tp[:], r(ones[0:1, 0:128]), r(rowr[:]),
                                     start=True, stop=True)
                    if dst is bvb:
                        nc.vector.tensor_copy(
                            dst[:], tp[:].rearrange("p (h d) -> p h d", h=H))
                    else:
                        nc.vector.tensor_copy(dst[:], tp[:])

            # ---- persistent activations ----
            kt_sb = [persist.tile([128, L], f32, tag=f"KT{c}", name=f"KT{c}")
                     for c in range(NCH)]
            v_sb = [persist.tile([128, H, DH + 1], bf16, tag=f"V{t}",
                        name=f"V{t}") for t in range(NT)]
            stage = [persist.tile([128, L], f32, tag=f"stage{c}", name=f"stage{c}")
                     for c in range(NCH)]

            with (
                tc.tile_pool(name="xin", bufs=XIN) as xin_pool,
                tc.tile_pool(name="qtg", bufs=2) as qtg_pool,
                tc.tile_pool(name="xt", bufs=XT) as xt_pool,
                tc.tile_pool(name="pexp", bufs=PEXP) as p_pool,
                tc.tile_pool(name="norm", bufs=1) as norm_pool,
                tc.tile_pool(name="osb", bufs=OSB) as o_pool,
            ):
                def emit_a_pieces(g):
                    """Return (qt_g, [thunk, ...]) -- pieces of the
                    transpose+projection work for token group g, woven
                    between attention head-pairs of the previous supertile."""
                    qt_g = [qtg_pool.tile([128, 512], f32, tag=f"qtg{c}",
                                          name=f"qtg{c}") for c in range(NCH)]
                    pieces = []
                    state = {}
                    for tname_, xdram_ in (("k", xk), ("v", xv), ("q", xq)):
                        pieces.append(
                            lambda tname=tname_, xdram=xdram_:
                            state.__setitem__(
                                tname, emit_a_transpose(g, xdram)))
                        pieces.append(
                            lambda tname=tname_: emit_a_proj(
                                g, tname, state[tname], qt_g))
                    return qt_g, pieces

                def emit_a_transpose(g, xdram):
                    if True:
                        xtiles = []
                        for j in range(4):
                            t0 = 4 * g + j
                            xt_in = xin_pool.tile([128, 512], f32, tag="xin",
                                                  name="xin")
                            nc.sync.dma_start(
                                xt_in[:], xdram[128 * t0:128 * (t0 + 1), :])
                            xtiles.append(xt_in)
                        xt_c = []
                        for c in range(NCH):
                            ps = ps512.tile([128, 512], f32, tag="ps512",
                                            name="ps512")
                            for j in range(4):
                                nc.tensor.transpose(
                                    ps[:, 128 * j:128 * (j + 1)],
                                    xtiles[j][:, 128 * c:128 * (c + 1)],
                                    ident[:],
                                )
                            sb = xt_pool.tile([128, 512], f32, tag="xt", name="xt")
                            nc.vector.tensor_copy(r(sb[:]), ps[:])
                            xt_c.append(sb)
                        return xt_c

                def emit_a_proj(g, tname, xt_c, qt_g):
                    if True:
                        if tname in ("q", "k"):
                            for co in range(NCH):
                                pp = ps512.tile([128, 512], f32, tag="ps512",
                                                name="ps512")
                                for ci in range(NCH):
                                    nc.tensor.matmul(
                                        pp[:],
                                        r(w_sb[tname][
                                            :, ci, 128 * co:128 * (co + 1)]),
                                        r(xt_c[ci][:]),
                                        start=(ci == 0), stop=(ci == NCH - 1),
                                    )
                                if tname == "q":
                                    nc.vector.tensor_scalar_add(
                                        r(qt_g[co][:]), pp[:],
                                        bcol["q"][:, co:co + 1])
                                else:
                                    nc.vector.tensor_scalar_add(
                                        r(kt_sb[co][:, 512 * g:512 * (g + 1)]),
                                        pp[:], bcol["k"][:, co:co + 1])
                        else:
                            for j in range(4):
                                t0 = 4 * g + j
                                pv = ps512.tile([128, 512], f32, tag="ps512",
                                                name="ps512")
                                for ci in range(NCH):
                                    nc.tensor.matmul(
                                        pv[:],
                                        r(xt_c[ci][:, 128 * j:128 * (j + 1)]),
                                        r(w_sb["v"][:, ci, :]),
                                        start=(ci == 0), stop=(ci == NCH - 1),
                                    )
                                nc.vector.tensor_add(
                                    v_sb[t0][:, :, 0:DH],
                                    pv[:].rearrange("p (h d) -> p h d", h=H),
                                    bvb[:],
                                )
                                nc.gpsimd.memset(v_sb[t0][:, :, DH:DH + 1], 1.0)

                def emit_b_qt(qt, qt_g, weave=()):
                    weave = list(weave)
                    kmax = 4 * qt + 4
                    stg = norm_pool.tile([128, 1536], f32, tag="stg", name="stg")
                    for hp in range(H // 2):
                        # head pair (2hp, 2hp+1): score matmuls alternate
                        # PE row-halves (prow 0/64) -> array-level overlap
                        ch = hp
                        kth = kt_sb[ch]
                        qth = qt_g[ch]
                        po = [ops_pool.tile([65, 512], f32, tag="ops",
                                            name="ops") for _ in range(2)]
                        for c in range(kmax):
                            m = c - 4 * qt
                            js0 = 0 if m < 1 else (128 * m if m < 3 else 256)
                            jv0 = 0 if m < 1 else 128 * m
                            ps = sps2_pool.tile([128, 1024], f32, tag="sps2",
                                                name="sps2")
                            pt = p_pool.tile([128, 1024], bf16, tag="pexp",
                                             name="pexp")
                            for k in range(2):
                                prow = 64 * k
                                nc.tensor.matmul(
                                    ps[:, 512 * k + js0:512 * (k + 1)],
                                    r(kth[prow:prow + DH,
                                          128 * c:128 * (c + 1)]),
                                    r(qth[prow:prow + DH, js0:512]),
                                    start=True, stop=True,
                                )
                            if m < 0:
                                nc.scalar.activation(
                                    pt[:], ps[:],
                                    mybir.ActivationFunctionType.Exp,
                                    scale=0.125,
                                )
                            else:
                                for k in range(2):
                                    nc.scalar.activation(
                                        pt[:, 512 * k + jv0:512 * (k + 1)],
                                        ps[:, 512 * k + jv0:512 * (k + 1)],
                                        mybir.ActivationFunctionType.Exp,
                                        scale=0.125,
                                    )
                            if m >= 0:
                                for k in range(2):
                                    nc.vector.tensor_mul(
                                        pt[:, 512 * k + 128 * m:
                                           512 * k + 128 * (m + 1)],
                                        pt[:, 512 * k + 128 * m:
                                           512 * k + 128 * (m + 1)], tri[:])
                            for k in range(2):
                                nc.tensor.matmul(
                                    po[k][:, jv0:512],
                                    v_sb[c][:, 2 * hp + k, :],
                                    pt[:, 512 * k + jv0:512 * (k + 1)],
                                    start=(c == 0), stop=(c == kmax - 1),
                                )
                        for k in range(2):
                            h = 2 * hp + k
                            prow = 64 * k
                            nc.vector.tensor_copy(
                                r(stage[ch][prow:prow + DH,
                                            512 * qt:512 * (qt + 1)]),
                                po[k][0:DH, :])
                            nc.vector.tensor_copy(
                                stg[32 * (h % 3):32 * (h % 3) + 1,
                                    512 * (h // 3):512 * (h // 3) + 512],
                                po[k][DH:DH + 1, :])
                        if weave and hp >= 1:
                            weave.pop(0)()
                            if weave:
                                weave.pop(0)()
                    rstg = norm_pool.tile([128, 1536], f32, tag="rstg",
                                          name="rstg")
                    nc.vector.reciprocal_approx_fast(out=rstg[:], in_=stg[:])
                    rbf = norm_pool.tile([128, 1536], bf16, tag="rbf", name="rbf")
                    nc.vector.tensor_copy(rbf[:], rstg[:])
                    for ch in range(NCH):
                        bcp = ps512.tile([128, 512], f32, tag="ps512",
                                         name="ps512")
                        for sub in range(2):
                            hh = 2 * ch + sub
                            pp0 = 32 * (hh % 3)
                            fo = 512 * (hh // 3)
                            nc.tensor.matmul(
                                bcp[64 * sub:64 * sub + 64, :],
                                ones_t[pp0:pp0 + 1, 0:64],
                                rbf[pp0:pp0 + 1, fo:fo + 512],
                                start=True, stop=True,
                            )
                        nc.vector.tensor_mul(
                            r(stage[ch][:, 512 * qt:512 * (qt + 1)]),
                            stage[ch][:, 512 * qt:512 * (qt + 1)],
                            bcp[:],
                        )
                    for i in range(4 * qt, 4 * qt + 4):
                        pout = ps512.tile([128, 512], f32, tag="ps512",
                                          name="ps512")
                        for ch in range(NCH):
                            nc.tensor.matmul(
                                pout[:],
                                r(stage[ch][:, 128 * i:128 * (i + 1)]),
                                r(w_sb["o"][:, ch, :]),
                                start=(ch == 0), stop=(ch == NCH - 1),
                            )
                        ot = o_pool.tile([128, 512], f32, tag="osb", name="osb")
                        nc.vector.tensor_add(ot[:], pout[:], bob[:])
                        nc.sync.dma_start(out[128 * i:128 * (i + 1), :], ot[:])
                    for w in weave:
                        w()

                def emit_body():
                    qt_g, pieces = emit_a_pieces(0)
                    for p in pieces:
                        p()
                    for g in range(NQ):
                        if g + 1 < NQ:
                            qt_next, weave = emit_a_pieces(g + 1)
                        else:
                            qt_next, weave = None, ()
                        emit_b_qt(g, qt_g, weave)
                        qt_g = qt_next

                if repeat > 1:
                    with tc.For_i(0, repeat, 1, hint_engines=(
                            mybir.EngineType.PE,
                            mybir.EngineType.DVE,
                            mybir.EngineType.Activation,
                            mybir.EngineType.SP,
                            mybir.EngineType.Pool)):
                        emit_body()
                else:
                    emit_body()

    nc.compile()
    return nc


def get_nc(repeat=1, cfg=None):
    key = f"nc{repeat}-{sorted((cfg or {}).items())}"
    if key not in _cached:
        _cached[key] = _build(repeat, cfg)
    return _cached[key]


def run(in_maps, trace=False, repeat=1, cfg=None, **kw):
    from concourse.bass_utils import run_bass_kernel_spmd

    nc = get_nc(repeat, cfg)
    return run_bass_kernel_spmd(nc, in_maps, list(range(B)), trace=trace, **kw)


def kernel(query, key, value, Wq, bq, Wk, bk, Wv, bv, Wo, bo):
    shared = {
        "Wq": np.ascontiguousarray(Wq, np.float32),
        "Wk": np.ascontiguousarray(Wk, np.float32),
        "Wv": np.ascontiguousarray(Wv, np.float32),
        "Wo": np.ascontiguousarray(Wo, np.float32),
        "bq": np.ascontiguousarray(bq, np.float32),
        "bk": np.ascontiguousarray(bk, np.float32),
        "bv": np.ascontiguousarray(bv, np.float32),
        "bo": np.ascontiguousarray(bo, np.float32),
    }
    in_maps = []
    for i in range(B):
        m = dict(shared)
        m["query"] = np.ascontiguousarray(query[i], np.float32)
        m["key"] = np.ascontiguousarray(key[i], np.float32)
        m["value"] = np.ascontiguousarray(value[i], np.float32)
        in_maps.append(m)
    res = run(in_maps)
    return np.stack([res.results[i]["out"] for i in range(B)], axis=0)
